# revision 1
# baseline (speedup 1.0000x reference)
import numpy as np

import concourse.bass as bass
from concourse import bacc
import concourse.mybir as mybir
import concourse.tile as tile
from concourse.bass_utils import run_bass_kernel_spmd

# ---- problem constants (hardcoded) ----
D = 256; NH = 8; NL = 4; NP = 4; DFF = 1024; BS = 8; NQ = 300
DH = D // NH  # 32
EPS = 1e-5
SPATIAL = np.array([[100, 150], [50, 75], [25, 38], [13, 19]], dtype=np.int64)
SIZES = (SPATIAL[:, 0] * SPATIAL[:, 1])
S = int(SIZES.sum())  # 19947
LSI = np.concatenate([[0], np.cumsum(SIZES)[:-1]]).astype(np.int64)
ST = S + 3  # table padded (pair windows read j, j+1; L3 slice needs +2); even

F32 = mybir.dt.float32
F32R = mybir.dt.float32r
BF16 = mybir.dt.bfloat16
U16 = mybir.dt.uint16
ALU = mybir.AluOpType
ACT = mybir.ActivationFunctionType
AX = mybir.AxisListType

QT = [(0, 128), (128, 128), (256, 44)]
GCH = [(i * 32, 32) for i in range(9)] + [(288, 16)]
WQPAD = 320
NSEL = 8
SUBW = 2560

_cache = {}
import os
SKIP = set(os.environ.get('KSKIP','').split(','))


USE_F32R = False


def _r(ap):
    return ap.bitcast(F32R) if USE_F32R else ap


def build_bass():
    nc = bacc.Bacc("TRN2", target_bir_lowering=False)
    tgtT = nc.dram_tensor("tgtT", [D, NQ], F32, kind="ExternalInput")
    posT = nc.dram_tensor("posT", [D, NQ], F32, kind="ExternalInput")
    memT = nc.dram_tensor("memT", [D, S], F32, kind="ExternalInput")
    refs = nc.dram_tensor("refs", [NQ, 8], F32, kind="ExternalInput")
    wqT = nc.dram_tensor("wqT", [D, D], F32, kind="ExternalInput")
    wkT = nc.dram_tensor("wkT", [D, D], F32, kind="ExternalInput")
    wvT = nc.dram_tensor("wvT", [D, D], F32, kind="ExternalInput")
    qb = nc.dram_tensor("qb", [D, 1], F32, kind="ExternalInput")
    kb = nc.dram_tensor("kb", [D, 1], F32, kind="ExternalInput")
    woT = nc.dram_tensor("woT", [D, D], F32, kind="ExternalInput")
    wob = nc.dram_tensor("wob", [D, 1], F32, kind="ExternalInput")
    wsoT = nc.dram_tensor("wsoT", [D, D], F32, kind="ExternalInput")
    wawT = nc.dram_tensor("wawT", [D, 128], F32, kind="ExternalInput")
    wvdT = nc.dram_tensor("wvdT", [D, D], F32, kind="ExternalInput")
    vdb = nc.dram_tensor("vdb", [D, 1], F32, kind="ExternalInput")
    wodT = nc.dram_tensor("wodT", [D, D], F32, kind="ExternalInput")
    wodb = nc.dram_tensor("wodb", [D, 1], F32, kind="ExternalInput")
    w1T = nc.dram_tensor("w1T", [D, DFF], F32, kind="ExternalInput")
    b1 = nc.dram_tensor("b1", [DFF, 1], F32, kind="ExternalInput")
    w2T = nc.dram_tensor("w2T", [DFF, D], F32, kind="ExternalInput")
    b2 = nc.dram_tensor("b2", [D, 1], F32, kind="ExternalInput")
    ln_gb = nc.dram_tensor("ln_gb", [D, 6], F32, kind="ExternalInput")
    consts = nc.dram_tensor("consts", [4, 256], F32, kind="ExternalInput")
    consts2 = nc.dram_tensor("consts2", [2, D], F32, kind="ExternalInput")
    ident_in = nc.dram_tensor("ident_in", [128, 128], F32, kind="ExternalInput")
    sel_in = nc.dram_tensor("sel_in", [NH * NSEL, 128 * NSEL], F32, kind="ExternalInput")
    outT = nc.dram_tensor("outT", [D, NQ], F32, kind="ExternalOutput")
    wdram = nc.dram_tensor("wdram", [NH, WQPAD * 64], F32)
    jdram = nc.dram_tensor("jdram", [81920], U16)

    with tile.TileContext(nc) as tc:
        import contextlib
        ctx = contextlib.ExitStack()
        with ctx:
            single = ctx.enter_context(tc.tile_pool(name="single", bufs=1))
            actp = ctx.enter_context(tc.tile_pool(name="actp", bufs=1))
            tmp = ctx.enter_context(tc.tile_pool(name="tmp", bufs=2))
            mpool = ctx.enter_context(tc.tile_pool(name="mpool", bufs=3))
            gpool = ctx.enter_context(tc.tile_pool(name="gpool", bufs=2))
            ppool = ctx.enter_context(tc.tile_pool(name="ppool", bufs=2, space="PSUM"))
            vpool = ctx.enter_context(tc.tile_pool(name="vpool", bufs=2, space="PSUM"))
            

            # stacked loader: DRAM [K*128, X] -> SBUF [128, K, X]
            def loadS(dram, rows, cols, pool=single):
                k = rows // 128
                nm = "w_" + dram.name
                t = pool.tile([128, k, cols], F32, name=nm, tag=nm)
                for i in range(k):
                    nc.sync.dma_start(out=t[:, i, :], in_=dram[i * 128:(i + 1) * 128, :])
                return t

            def loadC(dram, rows, cols, pool=single):  # small col tiles [rows<=128, cols]
                nm = "w_" + dram.name
                t = pool.tile([rows, cols], F32, name=nm, tag=nm)
                nc.sync.dma_start(out=t[:, :], in_=dram[:, :])
                return t

            sb_wq = loadS(wqT, D, D); sb_wk = loadS(wkT, D, D); sb_wv = loadS(wvT, D, D)
            sb_qb = loadS(qb, D, 1); sb_kb = loadS(kb, D, 1)
            sb_wo = loadS(woT, D, D); sb_wob = loadS(wob, D, 1)
            sb_wso = loadS(wsoT, D, D); sb_waw = loadS(wawT, D, 128)
            sb_wvd = single.tile([128, 2, D], F32, name="w_wvdT", tag="w_wvdT")
            for i in range(2):
                nc.sync.dma_start(out=sb_wvd[:, i, :].bitcast(F32R), in_=wvdT[i * 128:(i + 1) * 128, :].bitcast(F32R))
            sb_vdb = loadS(vdb, D, 1)
            sb_wod = loadS(wodT, D, D); sb_wodb = loadS(wodb, D, 1)
            sb_b1 = loadS(b1, DFF, 1)
            sb_b2 = loadS(b2, D, 1)
            sb_lngb = loadS(ln_gb, D, 6)
            sb_ident = loadC(ident_in, 128, 128)
            sb_sel = loadC(sel_in, NH * NSEL, 128 * NSEL)

            sb_consts = single.tile([128, 4, 256], F32)
            cap = consts[:, :]
            nc.sync.dma_start(out=sb_consts[:, :, :],
                              in_=bass.AP(tensor=cap.tensor, offset=cap.offset, ap=[[0, 128], [256, 4], [1, 256]]))
            SCLr = sb_consts[:, 0, :]; CM2r = sb_consts[:, 1, :]
            WLr = sb_consts[:, 0, :128]; LSIr = sb_consts[:, 2, :128]
            AWBr = sb_consts[:, 3, :128]
            sb_consts2 = single.tile([128, 2, D], F32)
            c2 = consts2[:, :]
            nc.sync.dma_start(out=sb_consts2[:, :, :],
                              in_=bass.AP(tensor=c2.tensor, offset=c2.offset, ap=[[0, 128], [D, 2], [1, D]]))
            SOBr = sb_consts2[:, 0, :]; VBSr = sb_consts2[:, 1, :]

            ones_col = single.tile([128, 1], F32)
            nc.vector.memset(ones_col[:, :], 1.0)
            ones_row = single.tile([1, 128], F32)
            nc.vector.memset(ones_row[:, :], 1.0)

            # activations stacked [128, 2, NQ]
            def newact():
                return actp.tile([128, 2, NQ], F32, tag="acts", name="acts", bufs=3)

            sb_tgt = actp.tile([128, 2, NQ], F32)
            sb_pos = actp.tile([128, 2, NQ], F32)
            for i in range(2):
                nc.sync.dma_start(out=sb_tgt[:, i, :], in_=tgtT[i * 128:(i + 1) * 128, :])
                nc.sync.dma_start(out=sb_pos[:, i, :], in_=posT[i * 128:(i + 1) * 128, :])

            def layer_norm(xT, gi, bi, out_t):
                ps_s = ppool.tile([1, NQ], F32, tag="ps", name="ps_s")
                ps_q = ppool.tile([1, NQ], F32, tag="ps", name="ps_q")
                for kk in range(2):
                    nc.tensor.matmul(ps_s[:, :], ones_col[:, :], xT[:, kk, :], start=(kk == 0), stop=(kk == 1))
                for kk in range(2):
                    sq = tmp.tile([128, NQ], F32, tag="lnsq", name="sq", bufs=1)
                    nc.vector.tensor_tensor(out=sq[:, :], in0=xT[:, kk, :], in1=xT[:, kk, :], op=ALU.mult)
                    nc.tensor.matmul(ps_q[:, :], ones_col[:, :], sq[:, :], start=(kk == 0), stop=(kk == 1))
                mean = tmp.tile([1, NQ], F32, tag="lnrow", bufs=4)
                nc.vector.tensor_scalar(out=mean[:, :], in0=ps_s[:, :], scalar1=1.0 / D, scalar2=None, op0=ALU.mult)
                var = tmp.tile([1, NQ], F32, tag="lnrow", bufs=4)
                nc.vector.tensor_scalar(out=var[:, :], in0=ps_q[:, :], scalar1=1.0 / D, scalar2=None, op0=ALU.mult)
                m2 = tmp.tile([1, NQ], F32, tag="lnrow", bufs=4)
                nc.vector.tensor_tensor(out=m2[:, :], in0=mean[:, :], in1=mean[:, :], op=ALU.mult)
                nc.vector.tensor_tensor(out=var[:, :], in0=var[:, :], in1=m2[:, :], op=ALU.subtract)
                nc.vector.tensor_scalar(out=var[:, :], in0=var[:, :], scalar1=EPS, scalar2=None, op0=ALU.add)
                nc.scalar.sqrt(out=var[:, :], in_=var[:, :])
                rstd = tmp.tile([1, NQ], F32, tag="lnrow", bufs=4)
                nc.vector.reciprocal(out=rstd[:, :], in_=var[:, :])
                nmr = tmp.tile([1, NQ], F32, tag="lnrow", bufs=4)
                nc.vector.tensor_tensor(out=nmr[:, :], in0=mean[:, :], in1=rstd[:, :], op=ALU.mult)
                nc.vector.tensor_scalar(out=nmr[:, :], in0=nmr[:, :], scalar1=-1.0, scalar2=None, op0=ALU.mult)
                ps_r = ppool.tile([128, NQ], F32, tag="ps", name="ps_r")
                ps_m = ppool.tile([128, NQ], F32, tag="ps", name="ps_m")
                nc.tensor.matmul(ps_r[:, :], ones_row[:, :], rstd[:, :], start=True, stop=True)
                nc.tensor.matmul(ps_m[:, :], ones_row[:, :], nmr[:, :], start=True, stop=True)
                for kk in range(2):
                    sl = slice(kk * 128, (kk + 1) * 128)
                    t1 = tmp.tile([128, NQ], F32, tag="lnt", name="t1", bufs=1)
                    nc.vector.tensor_tensor(out=t1[:, :], in0=xT[:, kk, :], in1=ps_r[:, :], op=ALU.mult)
                    nc.vector.tensor_tensor(out=t1[:, :], in0=t1[:, :], in1=ps_m[:, :], op=ALU.add)
                    nc.vector.tensor_scalar(out=out_t[:, kk, :], in0=t1[:, :],
                                            scalar1=sb_lngb[:, kk, gi:gi + 1], scalar2=sb_lngb[:, kk, bi:bi + 1],
                                            op0=ALU.mult, op1=ALU.add)
                return out_t

            # ================= self attention =================
            A = newact()
            for kk in range(2):
                nc.vector.tensor_tensor(out=A[:, kk, :], in0=sb_tgt[:, kk, :], in1=sb_pos[:, kk, :], op=ALU.add)

            def proj_T(src, w, bias_col, dst=None):
                if dst is None:
                    dst = newact()
                for mt in range(2):
                    ps = ppool.tile([128, NQ], F32, tag="ps")
                    for kk in range(2):
                        nc.tensor.matmul(ps[:, :], _r(w[:, kk, mt * 128:(mt + 1) * 128]),
                                         _r(src[:, kk, :]), start=(kk == 0), stop=(kk == 1))
                    if bias_col is not None:
                        nc.vector.tensor_scalar(out=dst[:, mt, :], in0=ps[:, :], scalar1=bias_col[:, mt, :],
                                                scalar2=None, op0=ALU.add)
                    else:
                        nc.vector.tensor_copy(out=dst[:, mt, :], in_=ps[:, :])
                return dst

            # packed q/k: 3 heads per tile at bases 0/32/64
            qk3 = {"q": [actp.tile([96, NQ], F32, name=f"q3_{i}") for i in range(3)],
                   "k": [actp.tile([96, NQ], F32, name=f"k3_{i}") for i in range(3)]}
            def qk_sl(which, h, cols):
                return qk3[which][h // 3][(h % 3) * DH:(h % 3) * DH + DH, cols]
            for w_, b_, which in ((sb_wq, sb_qb, "q"), (sb_wk, sb_kb, "k")):
                for mt in range(2):
                    ps = ppool.tile([128, NQ], F32, tag="ps", name="ps")
                    for kk in range(2):
                        nc.tensor.matmul(ps[:, :], _r(w_[:, kk, mt * 128:(mt + 1) * 128]),
                                         _r(A[:, kk, :]), start=(kk == 0), stop=(kk == 1))
                    for hh in range(4):
                        h = mt * 4 + hh
                        nc.vector.tensor_scalar(out=qk_sl(which, h, slice(None)),
                                                in0=ps[hh * DH:(hh + 1) * DH, :],
                                                scalar1=b_[hh * DH:(hh + 1) * DH, mt, :], scalar2=None, op0=ALU.add)
            v_nat = []
            for (q0, qn) in QT:
                ps = ppool.tile([128, D], F32, tag="ps")
                for kk in range(2):
                    nc.tensor.matmul(ps[:qn, :], _r(sb_tgt[:, kk, q0:q0 + qn]), _r(sb_wv[:, kk, :]),
                                     start=(kk == 0), stop=(kk == 1))
                vt = actp.tile([128, D], F32, tag="vnat", name="vt", bufs=3)
                nc.vector.tensor_tensor(out=vt[:qn, :], in0=ps[:qn, :], in1=VBSr[:qn, :], op=ALU.add)
                v_nat.append(vt)

            OT = newact()
            if 'attn' in SKIP:
                for kk in range(2):
                    nc.vector.memset(OT[:, kk, :], 0.0)
            for h in (range(NH) if 'attn' not in SKIP else []):
                kc = (h * DH) // 128
                ko = (h * DH) % 128
                attn = []
                for (q0, qn) in QT:
                    ps = ppool.tile([128, NQ], F32, tag="ps", name="ps")
                    nc.tensor.matmul(ps[:qn, :], _r(qk_sl("q", h, slice(q0, q0 + qn))), _r(qk_sl("k", h, slice(None))),
                                     start=True, stop=True)
                    # logits bounded (|x| < 2 for this model's scale); exp directly
                    ex = tmp.tile([128, NQ], F32, tag="sattn", name="ex", bufs=3)
                    nc.scalar.activation(out=ex[:qn, :], in_=ps[:qn, :], func=ACT.Exp)
                    sm = tmp.tile([128, 1], F32, tag="srow", bufs=3)
                    nc.vector.tensor_reduce(out=sm[:qn, :], in_=ex[:qn, :], op=ALU.add, axis=AX.X)
                    rc = tmp.tile([128, 1], F32, tag="srow", bufs=3)
                    nc.vector.reciprocal(out=rc[:qn, :], in_=sm[:qn, :])
                    nc.vector.tensor_scalar(out=ex[:qn, :], in0=ex[:qn, :], scalar1=rc[:qn, :], scalar2=None,
                                            op0=ALU.mult)
                    attn.append((ex, q0, qn))
                attnT = []
                for (k0, kn) in QT:
                    at = tmp.tile([128, NQ], F32, tag="sattnT", name="at", bufs=3)
                    for (aw_, q0, qn) in attn:
                        pst = ppool.tile([128, 128], F32, tag="pst", name="pst", bufs=2)
                        nc.tensor.transpose(pst[:kn, :qn], aw_[:qn, k0:k0 + kn], sb_ident[:qn, :qn])
                        nc.vector.tensor_copy(out=at[:kn, q0:q0 + qn], in_=pst[:kn, :qn])
                    attnT.append((at, k0, kn))
                ps_o = ppool.tile([DH, NQ], F32, tag="pst", name="ps_o", bufs=2)
                for ci, (at, k0, kn) in enumerate(attnT):
                    nc.tensor.matmul(ps_o[:, :], _r(v_nat[ci][:kn, h * DH:(h + 1) * DH]), _r(at[:kn, :]),
                                     start=(ci == 0), stop=(ci == 2))
                nc.vector.tensor_copy(out=OT[ko:ko + DH, kc, :], in_=ps_o[:, :])

            t2 = proj_T(OT, sb_wo, sb_wob)
            x1 = newact()
            for kk in range(2):
                nc.vector.tensor_tensor(out=x1[:, kk, :], in0=sb_tgt[:, kk, :], in1=t2[:, kk, :], op=ALU.add)
            x1n = layer_norm(x1, 0, 1, newact())  # norm2

            # ================= deformable attention =================
            vtab0 = single.tile([128, ST], BF16)
            vtab1 = single.tile([128, ST], BF16)
            nc.vector.memset(vtab0[:, S:], 0.0)
            nc.vector.memset(vtab1[:, S:], 0.0)
            SCH = 512
            ns_ch = (S + SCH - 1) // SCH
            if 'value' in SKIP:
                nc.vector.memset(vtab0[:, :], 0.0)
                nc.vector.memset(vtab1[:, :], 0.0)
            for si in (range(ns_ch) if 'value' not in SKIP else []):
                s0 = si * SCH
                sn = min(SCH, S - s0)
                snp = sn + (sn % 2)  # fp32r needs even moving dim
                mt_ = mpool.tile([128, 2, SCH], F32, tag="mem", name="mt_", bufs=2)
                if snp != sn:
                    for kk in range(2):
                        nc.vector.memset(mt_[:, kk, sn:snp], 0.0)
                for kk in range(2):
                    nc.sync.dma_start(out=mt_[:, kk, :sn].bitcast(F32R),
                                      in_=memT[kk * 128:(kk + 1) * 128, s0:s0 + sn].bitcast(F32R))
                for dp, vtab in ((0, vtab0), (1, vtab1)):
                    ps = vpool.tile([128, 1024], F32, tag="vwps", name="vps")
                    h0n = min(512, snp)
                    h1n = snp - h0n
                    for kk in range(2):
                        nc.tensor.matmul(ps[:, :h0n], sb_wvd[:, kk, dp * 128:(dp + 1) * 128].bitcast(F32R),
                                         mt_[:, kk, :h0n].bitcast(F32R), start=(kk == 0), stop=(kk == 1))
                    if h1n > 0:
                        for kk in range(2):
                            nc.tensor.matmul(ps[:, h0n:snp], sb_wvd[:, kk, dp * 128:(dp + 1) * 128].bitcast(F32R),
                                             mt_[:, kk, h0n:snp].bitcast(F32R), start=(kk == 0), stop=(kk == 1))
                    if si % 2 == 0:
                        nc.scalar.activation(out=vtab[:, s0:s0 + sn], in_=ps[:, :sn], func=ACT.Identity,
                                             bias=sb_vdb[:, dp, :])
                    else:
                        nc.vector.tensor_scalar(out=vtab[:, s0:s0 + sn], in0=ps[:, :sn],
                                                scalar1=sb_vdb[:, dp, :], scalar2=None, op0=ALU.add)

            q2 = newact()
            for kk in range(2):
                nc.vector.tensor_tensor(out=q2[:, kk, :], in0=x1n[:, kk, :], in1=sb_pos[:, kk, :], op=ALU.add)

            zt = single.tile([128, 160], F32, name="zt")
            nc.vector.memset(zt[:, :], 0.0)
            zp = zt[:, :]
            nc.sync.dma_start(out=bass.AP(tensor=wdram, offset=0, ap=[[1280, 128], [1, 1280]]),
                              in_=bass.AP(tensor=zp.tensor, offset=zp.offset,
                                          ap=[list(zp.ap[0]), [0, 8], [1, 160]]))
            idxs = single.tile([128, 640], U16)
            ji = [single.tile([128, 304], U16, name=f"ji{m}") for m in range(2)]
            for m in range(2):
                nc.vector.memset(ji[m][:, :], 0)
            zu = zt[:, :].bitcast(U16)
            nc.sync.dma_start(out=bass.AP(tensor=jdram, offset=0, ap=[[640, 128], [1, 640]]),
                              in_=bass.AP(tensor=zu.tensor, offset=zu.offset,
                                          ap=[list(zu.ap[0]), [0, 2], [1, 320]]))
            if 'samp' in SKIP:
                pass
            for ti, (q0, qn) in (list(enumerate(QT)) if 'samp' not in SKIP else []):
                rt = tmp.tile([128, 8], F32, tag="refs", name="rt", bufs=1)
                nc.sync.dma_start(out=rt[:qn, :], in_=refs[q0:q0 + qn, :])
                ps_off = ppool.tile([128, D], F32, tag="ps")
                for kk in range(2):
                    nc.tensor.matmul(ps_off[:qn, :], _r(q2[:, kk, q0:q0 + qn]), _r(sb_wso[:, kk, :]),
                                     start=(kk == 0), stop=(kk == 1))
                off = tmp.tile([128, D], F32, tag="off", name="off", bufs=1)
                nc.vector.tensor_tensor(out=off[:qn, :], in0=ps_off[:qn, :], in1=SOBr[:qn, :], op=ALU.add)
                ps_aw = ppool.tile([128, 128], F32, tag="pst", name="ps_aw", bufs=2)
                for kk in range(2):
                    nc.tensor.matmul(ps_aw[:qn, :], _r(q2[:, kk, q0:q0 + qn]), _r(sb_waw[:, kk, :]),
                                     start=(kk == 0), stop=(kk == 1))
                awl = tmp.tile([128, 128], F32, tag="aw")
                nc.vector.tensor_tensor(out=awl[:qn, :], in0=ps_aw[:qn, :], in1=AWBr[:qn, :], op=ALU.add)
                # aw logits bounded (|x| < 3 for this model's scale); exp directly
                nc.scalar.activation(out=awl[:qn, :], in_=awl[:qn, :], func=ACT.Exp)
                sm = tmp.tile([128, NH], F32, tag="awrow", bufs=3)
                nc.vector.tensor_reduce(out=sm[:qn, :], in_=awl[:qn, :].rearrange("p (h g) -> p h g", g=16),
                                        op=ALU.add, axis=AX.X)
                rc = tmp.tile([128, NH], F32, tag="awrow", bufs=3)
                nc.vector.reciprocal(out=rc[:qn, :], in_=sm[:qn, :])
                rca = rc[:qn, :]
                aw = tmp.tile([128, 128], F32, tag="aw")
                nc.vector.tensor_tensor(out=aw[:qn, :].rearrange("p (h g) -> p h g", g=16),
                                        in0=awl[:qn, :].rearrange("p (h g) -> p h g", g=16),
                                        in1=bass.AP(tensor=rca.tensor, offset=rca.offset,
                                                    ap=[list(rca.ap[0]), list(rca.ap[1]), [0, 16]]),
                                        op=ALU.mult)

                J = tmp.tile([128, D], F32, tag="J", name="J", bufs=1)
                Wt = tmp.tile([128, 512], F32, tag="Wt", name="Wt", bufs=1)
                ra = rt[:qn, :]
                p_ = tmp.tile([128, 256], F32, tag="pxy", name="p_", bufs=2)
                for xy in range(2):
                    refb = bass.AP(tensor=ra.tensor, offset=ra.offset + xy,
                                   ap=[list(ra.ap[0]), [0, NH], [2, NL], [0, NP]])
                    nc.vector.tensor_tensor(
                        out=p_[:qn, xy * 128:(xy + 1) * 128].rearrange("p (h l m) -> p h l m", h=NH, l=NL),
                        in0=refb,
                        in1=SCLr[:qn, xy * 128:(xy + 1) * 128].rearrange("p (h l m) -> p h l m", h=NH, l=NL),
                        op=ALU.mult)
                nc.vector.tensor_tensor(out=p_[:qn, :], in0=p_[:qn, :], in1=off[:qn, :], op=ALU.add)
                # shift +64 so trunc == floor; consts pre-shifted on host
                nc.vector.tensor_scalar(out=p_[:qn, :], in0=p_[:qn, :], scalar1=63.5, scalar2=None, op0=ALU.add)
                xi = tmp.tile([128, 256], mybir.dt.int32, tag="scr", name="xi", bufs=2)
                nc.vector.tensor_copy(out=xi[:qn, :], in_=p_[:qn, :])
                x0 = tmp.tile([128, 256], F32, tag="x0m", name="x0", bufs=1)
                nc.vector.tensor_copy(out=x0[:qn, :], in_=xi[:qn, :])
                # cast may trunc or round-to-nearest; fix up to floor either way
                gt_ = tmp.tile([128, 256], F32, tag="scr", name="gt_", bufs=2)
                nc.vector.tensor_tensor(out=gt_[:qn, :], in0=x0[:qn, :], in1=p_[:qn, :], op=ALU.is_gt)
                nc.vector.tensor_tensor(out=x0[:qn, :], in0=x0[:qn, :], in1=gt_[:qn, :], op=ALU.subtract)
                nc.vector.tensor_scalar(out=x0[:qn, :], in0=x0[:qn, :], scalar1=64.0, scalar2=None, op0=ALU.max)
                nc.vector.tensor_tensor(out=x0[:qn, :], in0=x0[:qn, :], in1=CM2r[:qn, :], op=ALU.min)
                w0 = tmp.tile([128, 256], F32, tag="w0m", name="w0", bufs=1)
                w1_ = tmp.tile([128, 256], F32, tag="w1m", name="w1_", bufs=1)
                dt_ = tmp.tile([128, 256], F32, tag="scr", name="dt_", bufs=2)
                nc.vector.tensor_tensor(out=dt_[:qn, :], in0=p_[:qn, :], in1=x0[:qn, :], op=ALU.subtract)
                ab0 = tmp.tile([128, 256], F32, tag="scr", name="ab0", bufs=2)
                nc.scalar.activation(out=ab0[:qn, :], in_=dt_[:qn, :], func=ACT.Abs)
                nc.vector.tensor_scalar(out=ab0[:qn, :], in0=ab0[:qn, :], scalar1=-1.0, scalar2=1.0,
                                        op0=ALU.mult, op1=ALU.add)
                nc.vector.tensor_scalar(out=w0[:qn, :], in0=ab0[:qn, :], scalar1=0.0, scalar2=None, op0=ALU.max)
                nc.vector.tensor_scalar(out=dt_[:qn, :], in0=dt_[:qn, :], scalar1=-1.0, scalar2=None, op0=ALU.add)
                nc.scalar.activation(out=ab0[:qn, :], in_=dt_[:qn, :], func=ACT.Abs)
                nc.vector.tensor_scalar(out=ab0[:qn, :], in0=ab0[:qn, :], scalar1=-1.0, scalar2=1.0,
                                        op0=ALU.mult, op1=ALU.add)
                nc.vector.tensor_scalar(out=w1_[:qn, :], in0=ab0[:qn, :], scalar1=0.0, scalar2=None, op0=ALU.max)
                (xx0, wx0, wx1) = (x0[:, :128], w0[:, :128], w1_[:, :128])
                (yy0, wy0, wy1) = (x0[:, 128:], w0[:, 128:], w1_[:, 128:])
                jb = tmp.tile([128, 128], F32, tag="jb", name="jb", bufs=1)
                nc.vector.tensor_tensor(out=jb[:qn, :], in0=yy0[:qn, :], in1=WLr[:qn, :], op=ALU.mult)
                nc.vector.tensor_tensor(out=jb[:qn, :], in0=jb[:qn, :], in1=xx0[:qn, :], op=ALU.add)
                nc.vector.tensor_tensor(out=jb[:qn, :], in0=jb[:qn, :], in1=LSIr[:qn, :], op=ALU.add)
                Jv = J[:qn, :].rearrange("p (f c) -> p f c", c=2)
                nc.vector.tensor_copy(out=Jv[:, :, 0], in_=jb[:qn, :])
                nc.vector.tensor_tensor(out=Jv[:, :, 1], in0=jb[:qn, :], in1=WLr[:qn, :], op=ALU.add)
                t_c = []
                for c, wyc in ((0, wy0), (1, wy1)):
                    tc_ = tmp.tile([128, 128], F32, tag=f"tc{c}", name="tc_", bufs=1)
                    nc.vector.tensor_tensor(out=tc_[:qn, :], in0=aw[:qn, :], in1=wyc[:qn, :], op=ALU.mult)
                    t_c.append(tc_)
                Wv4 = Wt[:qn, :].rearrange("p (f c s) -> p f c s", c=2, s=2)
                for c in range(2):
                    for sp, wxv in ((0, wx0), (1, wx1)):
                        nc.vector.tensor_tensor(out=Wv4[:, :, c, sp], in0=t_c[c][:qn, :], in1=wxv[:qn, :],
                                                op=ALU.mult)
                wa = Wt[:qn, :]
                nc.sync.dma_start(
                    out=bass.AP(tensor=wdram, offset=q0 * 16,
                                ap=[[16, qn], [WQPAD * 64, NH], [5120, 4], [4, 4], [2, 2], [1, 2]]),
                    in_=bass.AP(tensor=wa.tensor, offset=wa.offset,
                                ap=[list(wa.ap[0]), [64, NH], [16, 4], [4, 4], [2, 2], [1, 2]]))
                for m in range(2):
                    pst = ppool.tile([128, 128], F32, tag="pst", name="pst", bufs=2)
                    nc.tensor.transpose(pst[:, :qn], J[:qn, m * 128:(m + 1) * 128], sb_ident[:qn, :qn])
                    pv = pst[:, :qn].rearrange("p (a b) -> p a b", b=2)
                    nc.vector.tensor_copy(out=ji[m][:, q0 // 2:q0 // 2 + qn // 2], in_=pv[:, :, 0])
                    nc.vector.tensor_copy(out=ji[m][:, 152 + q0 // 2:152 + q0 // 2 + qn // 2], in_=pv[:, :, 1])

            for m in range(2):
                # ji rows = (h',l,p,c); free = (qm2, qh152). jdram [m][h'][qm2][p][c][l][qh160]
                for hp in range(4):
                    for qm2 in range(2):
                        jm = ji[m][hp * 32:(hp + 1) * 32, qm2 * 152:(qm2 + 1) * 152]
                        eng = nc.sync if (hp % 2 == 0) else nc.scalar
                        eng.dma_start(
                            out=bass.AP(tensor=jdram, offset=m * 40960 + hp * 10240 + qm2 * 5120,
                                        ap=[[160, 4], [1280, 4], [640, 2], [1, 152]]),
                            in_=bass.AP(tensor=jm.tensor, offset=jm.offset,
                                        ap=[list(jm.ap[0]), [1, 152]]))
            iap = idxs[:, :]
            nc.sync.dma_start(
                out=bass.AP(tensor=iap.tensor, offset=iap.offset, ap=[list(iap.ap[0]), [1, 640]]),
                in_=bass.AP(tensor=jdram, offset=0, ap=[[1, 81920]]))

            wsb = single.tile([NH * NSEL, SUBW], F32)
            nc.sync.dma_start(out=wsb[:, :],
                              in_=bass.AP(tensor=wdram, offset=0, ap=[[WQPAD * 64, NH], [SUBW, NSEL], [1, SUBW]]))

            ODl = [[actp.tile([128, 320], F32, name=f"OD{i}_{l}") for l in range(NL)] for i in range(2)]
            LVB = [(0, 15002), (15000, 3752), (18750, 952), (19700, 250)]
            for lv in (range(NL) if 'gather' not in SKIP else []):
                b0, blen = LVB[lv]
                for gi in range(5):
                    sl0 = gi * 32
                    gts = []
                    for dp, vtab in ((0, vtab0), (1, vtab1)):
                        gt = gpool.tile([128, 512, 2], BF16, tag="gather", name="gt", bufs=2)
                        nc.gpsimd.indirect_copy(
                            out=gt[:, :, :],
                            data=vtab[:, b0:b0 + blen].rearrange("p (a b) -> p a b", b=2),
                            idxs=idxs[:, lv * 160 + sl0:lv * 160 + sl0 + 32],
                            i_know_ap_gather_is_preferred=True)
                        gts.append(gt)
                    psw = vpool.tile([128, 1024], F32, tag="vwps", name="psw")
                    for half in range(2):
                        el0 = lv * 5120 + (sl0 + half * 16) * 32
                        sub = el0 // SUBW
                        eoff = el0 % SUBW
                        nc.tensor.matmul(psw[:, half * 512:(half + 1) * 512],
                                         _r(sb_sel[:, sub * 128:(sub + 1) * 128]),
                                         _r(wsb[:, eoff:eoff + 512]), start=True, stop=True)
                    for dp in range(2):
                        gfl = gts[dp][:, :, :].rearrange("p a b -> p (a b)")
                        prod = mpool.tile([128, 1024], F32, tag="prod", name="prod", bufs=2)
                        nc.vector.tensor_tensor(out=prod[:, :], in0=gfl, in1=psw[:, :], op=ALU.mult)
                        nc.vector.tensor_reduce(out=ODl[dp][lv][:, sl0 * 2:sl0 * 2 + 64],
                                                in_=prod[:, :].rearrange("p (a b) -> p a b", b=16),
                                                op=ALU.add, axis=AX.X)
            if 'gather' in SKIP:
                for dp in range(2):
                    for lv in range(NL):
                        nc.vector.memset(ODl[dp][lv][:, :], 0.0)
            t2d = newact()
            for mt in range(2):
                ps = ppool.tile([128, NQ], F32, tag="ps")
                for ci, (kk, lv) in enumerate([(k_, l_) for k_ in range(2) for l_ in range(NL)]):
                    nc.tensor.matmul(ps[:, :], _r(sb_wod[:, kk, mt * 128:(mt + 1) * 128]),
                                     _r(ODl[kk][lv][:, :NQ]), start=(ci == 0), stop=(ci == 7))
                nc.vector.tensor_scalar(out=t2d[:, mt, :], in0=ps[:, :], scalar1=sb_wodb[:, mt, :],
                                        scalar2=None, op0=ALU.add)
            x2 = newact()
            for kk in range(2):
                nc.vector.tensor_tensor(out=x2[:, kk, :], in0=x1n[:, kk, :], in1=t2d[:, kk, :], op=ALU.add)
            x2n = layer_norm(x2, 2, 3, newact())  # norm1

            # ================= FFN =================
            h1 = actp.tile([128, 8, NQ], F32)
            if 'ffn' in SKIP:
                for mt in range(8):
                    nc.vector.memset(h1[:, mt, :], 0.0)
            for mt in (range(8) if 'ffn' not in SKIP else []):
                ps = ppool.tile([128, NQ], F32, tag="ps")
                for kk in range(2):
                    wt1 = mpool.tile([128, 128], F32, tag="w2s", name="wt1", bufs=2)
                    nc.sync.dma_start(out=wt1[:, :], in_=w1T[kk * 128:(kk + 1) * 128, mt * 128:(mt + 1) * 128])
                    nc.tensor.matmul(ps[:, :], _r(wt1[:, :]), _r(x2n[:, kk, :]),
                                     start=(kk == 0), stop=(kk == 1))
                nc.scalar.activation(out=h1[:, mt, :], in_=ps[:, :], func=ACT.Relu, bias=sb_b1[:, mt, :])
            t2f = newact()
            for mt in range(2):
                ps = ppool.tile([128, NQ], F32, tag="ps")
                for kk in range(8):
                    wt2 = mpool.tile([128, 128], F32, tag="w2s", name="w2s", bufs=2)
                    nc.sync.dma_start(out=wt2[:, :], in_=w2T[kk * 128:(kk + 1) * 128, mt * 128:(mt + 1) * 128])
                    nc.tensor.matmul(ps[:, :], _r(wt2[:, :]), _r(h1[:, kk, :]),
                                     start=(kk == 0), stop=(kk == 7))
                nc.vector.tensor_scalar(out=t2f[:, mt, :], in0=ps[:, :], scalar1=sb_b2[:, mt, :],
                                        scalar2=None, op0=ALU.add)
            x3 = newact()
            for kk in range(2):
                nc.vector.tensor_tensor(out=x3[:, kk, :], in0=x2n[:, kk, :], in1=t2f[:, kk, :], op=ALU.add)
            x3n = layer_norm(x3, 4, 5, newact())  # norm3
            for kk in range(2):
                nc.sync.dma_start(out=outT[kk * 128:(kk + 1) * 128, :], in_=x3n[:, kk, :])

    nc.compile()
    return nc


def _perm_so():
    # samp_off_w rows are (h, l, p, xy); reorder to (xy, h, l, p)
    return np.array([((h * NL + l) * NP + p) * 2 + xy
                     for xy in range(2) for h in range(NH) for l in range(NL) for p in range(NP)])


def _host_prep(inputs):
    f = lambda x: np.ascontiguousarray(np.asarray(x, dtype=np.float32))
    in_w = f(inputs["in_proj_w"]); in_b = f(inputs["in_proj_b"])
    qw, kw, vw = in_w[:D], in_w[D:2 * D], in_w[2 * D:]
    qb_, kb_, vb_ = in_b[:D], in_b[D:2 * D], in_b[2 * D:]
    sc = 1.0 / np.sqrt(DH)
    perm = np.array([h * DH + dp * 16 + r for dp in range(2) for h in range(NH) for r in range(16)])
    shared = {
        "wqT": (qw * sc).T, "wkT": kw.T, "wvT": vw.T,
        "qb": (qb_ * sc)[:, None], "kb": kb_[:, None],
        "woT": f(inputs["out_proj_w"]).T, "wob": f(inputs["out_proj_b"])[:, None],
        "wsoT": f(inputs["samp_off_w"])[_perm_so()].T, "wawT": f(inputs["attn_wt_w"]).T,
        "wvdT": f(inputs["value_w"])[perm].T, "vdb": f(inputs["value_b"])[perm][:, None],
        "wodT": f(inputs["outp_w"]).T[perm], "wodb": f(inputs["outp_b"])[:, None],
        "w1T": f(inputs["lin1_w"]).T, "b1": f(inputs["lin1_b"])[:, None],
        "w2T": f(inputs["lin2_w"]).T, "b2": f(inputs["lin2_b"])[:, None],
        "ln_gb": np.stack([f(inputs["norm2_g"]), f(inputs["norm2_b"]),
                           f(inputs["norm1_g"]), f(inputs["norm1_b"]),
                           f(inputs["norm3_g"]), f(inputs["norm3_b"])], axis=1),
        "ident_in": np.eye(128, dtype=np.float32),
    }
    Wv_ = SPATIAL[:, 1].astype(np.float32); Hv_ = SPATIAL[:, 0].astype(np.float32)
    row = lambda vals: np.tile(np.repeat(vals, NP), NH)
    lsi_adj = -64.0 * Wv_ - 64.0  # level-local indices
    pad128 = np.zeros(128, np.float32)
    shared["consts"] = np.stack([
        np.concatenate([row(Wv_), row(Hv_)]),
        np.concatenate([row(Wv_ + 62), row(Hv_ + 62)]),
        np.concatenate([row(lsi_adj), pad128]),
        np.concatenate([f(inputs["attn_wt_b"]), pad128])]).astype(np.float32)
    shared["consts2"] = np.stack([f(inputs["samp_off_b"])[_perm_so()], vb_]).astype(np.float32)
    sel = np.zeros((NH * NSEL, NSEL * 128), dtype=np.float32)
    for s_ in range(NSEL):
        for p in range(128):
            sel[(p // 16) * NSEL + s_, s_ * 128 + p] = 1.0
    shared["sel_in"] = sel
    shared = {k: np.ascontiguousarray(np.asarray(v, np.float32)) for k, v in shared.items()}
    per_core = []
    for b in range(BS):
        m = dict(shared)
        m["tgtT"] = np.ascontiguousarray(f(inputs["tgt"][b]).T)
        m["posT"] = np.ascontiguousarray(f(inputs["tgt_query_pos"][b]).T)
        m["memT"] = np.ascontiguousarray(f(inputs["memory"][b]).T)
        m["refs"] = np.ascontiguousarray(f(inputs["tgt_reference_points"][b]).reshape(NQ, 8))
        per_core.append(m)
    return per_core


def kernel(**inputs) -> np.ndarray:
    if "nc" not in _cache:
        _cache["nc"] = build_bass()
    nc = _cache["nc"]
    in_maps = _host_prep(inputs)
    res = run_bass_kernel_spmd(nc, in_maps, core_ids=list(range(BS)))
    out = np.stack([np.ascontiguousarray(r["outT"].T) for r in res.results])
    return out.astype(np.float32)



# revision 45
# speedup vs baseline: 1.7461x; 1.7461x over previous
import numpy as np

import concourse.bass as bass
from concourse import bacc
import concourse.mybir as mybir
import concourse.tile as tile
from concourse.bass_utils import run_bass_kernel_spmd

# ---- problem constants (hardcoded) ----
D = 256; NH = 8; NL = 4; NP = 4; DFF = 1024; BS = 8; NQ = 300
DH = D // NH  # 32
EPS = 1e-5
SPATIAL = np.array([[100, 150], [50, 75], [25, 38], [13, 19]], dtype=np.int64)
SIZES = (SPATIAL[:, 0] * SPATIAL[:, 1])
S = int(SIZES.sum())  # 19947
LSI = np.concatenate([[0], np.cumsum(SIZES)[:-1]]).astype(np.int64)
ST = S + 3  # table padded (pair windows read j, j+1; L3 slice needs +2); even

F32 = mybir.dt.float32
F32R = mybir.dt.float32r
BF16 = mybir.dt.bfloat16
U16 = mybir.dt.uint16
ALU = mybir.AluOpType
ACT = mybir.ActivationFunctionType
AX = mybir.AxisListType

QT = [(0, 128), (128, 128), (256, 44)]
GCH = [(i * 32, 32) for i in range(9)] + [(288, 16)]
WQPAD = 320
NSEL = 8
SUBW = 2560

_cache = {}
import os
SKIP = set(os.environ.get('KSKIP','').split(','))


USE_F32R = False


def _r(ap):
    return ap.bitcast(F32R) if USE_F32R else ap


def build_bass():
    nc = bacc.Bacc("TRN2", target_bir_lowering=False)
    tgtT = nc.dram_tensor("tgtT", [D, NQ], BF16, kind="ExternalInput")
    posT = nc.dram_tensor("posT", [D, NQ], BF16, kind="ExternalInput")
    memTb = nc.dram_tensor("memTb", [D, S], BF16, kind="ExternalInput")
    refs = nc.dram_tensor("refs", [NQ, 8], F32, kind="ExternalInput")
    wqT = nc.dram_tensor("wqT", [D, D], BF16, kind="ExternalInput")
    wkT = nc.dram_tensor("wkT", [D, D], BF16, kind="ExternalInput")
    wvT = nc.dram_tensor("wvT", [D, D], BF16, kind="ExternalInput")
    qb = nc.dram_tensor("qb", [D, 1], F32, kind="ExternalInput")
    kb = nc.dram_tensor("kb", [D, 1], F32, kind="ExternalInput")
    woT = nc.dram_tensor("woT", [D, D], BF16, kind="ExternalInput")
    wob = nc.dram_tensor("wob", [D, 1], F32, kind="ExternalInput")
    wsoT = nc.dram_tensor("wsoT", [D, D], BF16, kind="ExternalInput")
    wawT = nc.dram_tensor("wawT", [D, 128], BF16, kind="ExternalInput")
    wvdTb = nc.dram_tensor("wvdTb", [D, D], BF16, kind="ExternalInput")
    vdb = nc.dram_tensor("vdb", [D, 1], F32, kind="ExternalInput")
    wodT = nc.dram_tensor("wodT", [D, D], BF16, kind="ExternalInput")
    wodb = nc.dram_tensor("wodb", [D, 1], F32, kind="ExternalInput")
    w1T = nc.dram_tensor("w1T", [D, DFF], BF16, kind="ExternalInput")
    b1 = nc.dram_tensor("b1", [DFF, 1], F32, kind="ExternalInput")
    w2T = nc.dram_tensor("w2T", [DFF, D], BF16, kind="ExternalInput")
    b2 = nc.dram_tensor("b2", [D, 1], F32, kind="ExternalInput")
    ln_gb = nc.dram_tensor("ln_gb", [D, 6], F32, kind="ExternalInput")
    consts = nc.dram_tensor("consts", [4, 256], F32, kind="ExternalInput")
    consts2 = nc.dram_tensor("consts2", [2, D], F32, kind="ExternalInput")
    ident_in = nc.dram_tensor("ident_in", [128, 128], F32, kind="ExternalInput")
    identb_in = nc.dram_tensor("identb_in", [128, 128], BF16, kind="ExternalInput")
    sel_in = nc.dram_tensor("sel_in", [NH * NSEL, 128 * NSEL], BF16, kind="ExternalInput")
    outT = nc.dram_tensor("outT", [D, NQ], F32, kind="ExternalOutput")
    wdram = nc.dram_tensor("wdram", [NH, WQPAD * 64], BF16)
    jdram = nc.dram_tensor("jdram", [163840], mybir.dt.int16)

    with tile.TileContext(nc) as tc:
        import contextlib
        ctx = contextlib.ExitStack()
        with ctx:
            single = ctx.enter_context(tc.tile_pool(name="single", bufs=1))
            actp = ctx.enter_context(tc.tile_pool(name="actp", bufs=1))
            tmp = ctx.enter_context(tc.tile_pool(name="tmp", bufs=2))
            mpool = ctx.enter_context(tc.tile_pool(name="mpool", bufs=3))
            gpool = ctx.enter_context(tc.tile_pool(name="gpool", bufs=2))
            ppool = ctx.enter_context(tc.tile_pool(name="ppool", bufs=2, space="PSUM"))
            vpool = ctx.enter_context(tc.tile_pool(name="vpool", bufs=2, space="PSUM"))
            

            # stacked loader: DRAM [K*128, X] -> SBUF [128, K, X]
            def loadS(dram, rows, cols, pool=single, dt=F32):
                k = rows // 128
                nm = "w_" + dram.name
                t = pool.tile([128, k, cols], dt, name=nm, tag=nm)
                for i in range(k):
                    nc.sync.dma_start(out=t[:, i, :], in_=dram[i * 128:(i + 1) * 128, :])
                return t

            def loadC(dram, rows, cols, pool=single, dt=F32):  # small col tiles [rows<=128, cols]
                nm = "w_" + dram.name
                t = pool.tile([rows, cols], dt, name=nm, tag=nm)
                nc.sync.dma_start(out=t[:, :], in_=dram[:, :])
                return t

            sb_wq = loadS(wqT, D, D, dt=BF16); sb_wk = loadS(wkT, D, D, dt=BF16); sb_wv = loadS(wvT, D, D, dt=BF16)
            sb_qb = loadS(qb, D, 1); sb_kb = loadS(kb, D, 1)
            sb_wo = loadS(woT, D, D, dt=BF16); sb_wob = loadS(wob, D, 1)
            sb_wso = loadS(wsoT, D, D, dt=BF16); sb_waw = loadS(wawT, D, 128, dt=BF16)
            sb_wvd = single.tile([128, 2, D], BF16, name="w_wvdT", tag="w_wvdT")
            nc.sync.dma_start(out=sb_wvd[:, :, :],
                              in_=bass.AP(tensor=wvdTb, offset=0,
                                          ap=[[D, 128], [128 * D, 2], [1, D]]))
            sb_vdb = loadS(vdb, D, 1)
            sb_wod = loadS(wodT, D, D, dt=BF16); sb_wodb = loadS(wodb, D, 1)
            sb_b1 = loadS(b1, DFF, 1)
            sb_b2 = loadS(b2, D, 1)
            sb_lngb = loadS(ln_gb, D, 6)
            sb_ident = loadC(ident_in, 128, 128)
            sb_identb = loadC(identb_in, 128, 128, dt=BF16)
            sb_sel = loadC(sel_in, NH * NSEL, 128 * NSEL, dt=BF16)

            sb_consts = single.tile([128, 4, 256], F32)
            cap = consts[:, :]
            nc.sync.dma_start(out=sb_consts[:, :, :],
                              in_=bass.AP(tensor=cap.tensor, offset=cap.offset, ap=[[0, 128], [256, 4], [1, 256]]))
            SCLr = sb_consts[:, 0, :]; CM2r = sb_consts[:, 1, :]
            WLr = sb_consts[:, 0, :128]; LSIr = sb_consts[:, 2, :128]
            AWBr = sb_consts[:, 3, :128]
            sb_consts2 = single.tile([128, 2, D], F32)
            c2 = consts2[:, :]
            nc.sync.dma_start(out=sb_consts2[:, :, :],
                              in_=bass.AP(tensor=c2.tensor, offset=c2.offset, ap=[[0, 128], [D, 2], [1, D]]))
            SOBr = sb_consts2[:, 0, :]; VBSr = sb_consts2[:, 1, :]

            ones_col = single.tile([128, 1], BF16)
            nc.vector.memset(ones_col[:, :], 1.0)
            ones_row = single.tile([1, 128], BF16)
            nc.vector.memset(ones_row[:, :], 1.0)

            # activations stacked [128, 2, NQ]
            def newact():
                return actp.tile([128, 2, NQ], BF16, tag="acts", name="acts", bufs=3)

            sb_tgt = actp.tile([128, 2, NQ], BF16)
            sb_pos = actp.tile([128, 2, NQ], BF16)
            for i in range(2):
                nc.sync.dma_start(out=sb_tgt[:, i, :], in_=tgtT[i * 128:(i + 1) * 128, :])
                nc.sync.dma_start(out=sb_pos[:, i, :], in_=posT[i * 128:(i + 1) * 128, :])

            def layer_norm(xT, gi, bi, out_t):
                ps_s = ppool.tile([1, NQ], F32, tag="ps", name="ps_s")
                ps_q = ppool.tile([1, NQ], F32, tag="ps", name="ps_q")
                for kk in range(2):
                    nc.tensor.matmul(ps_s[:, :], ones_col[:, :], xT[:, kk, :], start=(kk == 0), stop=(kk == 1))
                for kk in range(2):
                    sq = tmp.tile([128, NQ], BF16, tag="lnsq", name="sq", bufs=1)
                    nc.vector.tensor_tensor(out=sq[:, :], in0=xT[:, kk, :], in1=xT[:, kk, :], op=ALU.mult)
                    nc.tensor.matmul(ps_q[:, :], ones_col[:, :], sq[:, :], start=(kk == 0), stop=(kk == 1))
                mean = tmp.tile([1, NQ], F32, tag="lnrow", bufs=4)
                nc.vector.tensor_scalar(out=mean[:, :], in0=ps_s[:, :], scalar1=1.0 / D, scalar2=None, op0=ALU.mult)
                var = tmp.tile([1, NQ], F32, tag="lnrow", bufs=4)
                nc.vector.tensor_scalar(out=var[:, :], in0=ps_q[:, :], scalar1=1.0 / D, scalar2=None, op0=ALU.mult)
                m2 = tmp.tile([1, NQ], F32, tag="lnrow", bufs=4)
                nc.vector.tensor_tensor(out=m2[:, :], in0=mean[:, :], in1=mean[:, :], op=ALU.mult)
                nc.vector.tensor_tensor(out=var[:, :], in0=var[:, :], in1=m2[:, :], op=ALU.subtract)
                nc.vector.tensor_scalar(out=var[:, :], in0=var[:, :], scalar1=EPS, scalar2=None, op0=ALU.add)
                nc.scalar.sqrt(out=var[:, :], in_=var[:, :])
                rstd = tmp.tile([1, NQ], BF16, tag="lnrowb", bufs=2)
                with nc.allow_low_precision(reason="bf16 rstd, tol 2e-2"):
                    nc.vector.reciprocal(out=rstd[:, :], in_=var[:, :])
                nmr = tmp.tile([1, NQ], BF16, tag="lnrowb", bufs=2)
                nc.vector.tensor_tensor(out=nmr[:, :], in0=mean[:, :], in1=rstd[:, :], op=ALU.mult)
                nc.vector.tensor_scalar(out=nmr[:, :], in0=nmr[:, :], scalar1=-1.0, scalar2=None, op0=ALU.mult)
                ps_r = ppool.tile([128, NQ], F32, tag="ps", name="ps_r")
                ps_m = ppool.tile([128, NQ], F32, tag="ps", name="ps_m")
                nc.tensor.matmul(ps_r[:, :], ones_row[:, :], rstd[:, :], start=True, stop=True)
                nc.tensor.matmul(ps_m[:, :], ones_row[:, :], nmr[:, :], start=True, stop=True)
                for kk in range(2):
                    sl = slice(kk * 128, (kk + 1) * 128)
                    t1 = tmp.tile([128, NQ], F32, tag="lnt", name="t1", bufs=1)
                    nc.vector.tensor_tensor(out=t1[:, :], in0=xT[:, kk, :], in1=ps_r[:, :], op=ALU.mult)
                    nc.vector.tensor_tensor(out=t1[:, :], in0=t1[:, :], in1=ps_m[:, :], op=ALU.add)
                    nc.vector.tensor_scalar(out=out_t[:, kk, :], in0=t1[:, :],
                                            scalar1=sb_lngb[:, kk, gi:gi + 1], scalar2=sb_lngb[:, kk, bi:bi + 1],
                                            op0=ALU.mult, op1=ALU.add)
                return out_t

            # ================= self attention =================
            A = newact()
            for kk in range(2):
                nc.vector.tensor_tensor(out=A[:, kk, :], in0=sb_tgt[:, kk, :], in1=sb_pos[:, kk, :], op=ALU.add)

            def proj_T(src, w, bias_col, dst=None):
                if dst is None:
                    dst = newact()
                for mt in range(2):
                    ps = ppool.tile([128, NQ], F32, tag="ps")
                    for kk in range(2):
                        nc.tensor.matmul(ps[:, :], _r(w[:, kk, mt * 128:(mt + 1) * 128]),
                                         _r(src[:, kk, :]), start=(kk == 0), stop=(kk == 1))
                    if bias_col is not None:
                        nc.vector.tensor_scalar(out=dst[:, mt, :], in0=ps[:, :], scalar1=bias_col[:, mt, :],
                                                scalar2=None, op0=ALU.add)
                    else:
                        nc.vector.tensor_copy(out=dst[:, mt, :], in_=ps[:, :])
                return dst

            # packed q/k: 3 heads per tile at bases 0/32/64
            qk3 = {"q": [actp.tile([96, NQ], BF16, name=f"q3_{i}") for i in range(3)],
                   "k": [actp.tile([96, NQ], BF16, name=f"k3_{i}") for i in range(3)]}
            def qk_sl(which, h, cols):
                return qk3[which][h // 3][(h % 3) * DH:(h % 3) * DH + DH, cols]
            for w_, b_, which in ((sb_wq, sb_qb, "q"), (sb_wk, sb_kb, "k")):
                for mt in range(2):
                    ps = ppool.tile([128, NQ], F32, tag="ps", name="ps")
                    for kk in range(2):
                        nc.tensor.matmul(ps[:, :], _r(w_[:, kk, mt * 128:(mt + 1) * 128]),
                                         _r(A[:, kk, :]), start=(kk == 0), stop=(kk == 1))
                    for hh in range(4):
                        h = mt * 4 + hh
                        nc.scalar.activation(out=qk_sl(which, h, slice(None)),
                                             in_=ps[hh * DH:(hh + 1) * DH, :],
                                             func=ACT.Identity, bias=b_[hh * DH:(hh + 1) * DH, mt, :])
            v_nat = []
            for (q0, qn) in QT:
                ps = ppool.tile([128, D], F32, tag="ps")
                for kk in range(2):
                    nc.tensor.matmul(ps[:qn, :], _r(sb_tgt[:, kk, q0:q0 + qn]), _r(sb_wv[:, kk, :]),
                                     start=(kk == 0), stop=(kk == 1))
                vt = actp.tile([128, D], BF16, tag="vnat", name="vt", bufs=3)
                nc.vector.tensor_tensor(out=vt[:qn, :], in0=ps[:qn, :], in1=VBSr[:qn, :], op=ALU.add)
                v_nat.append(vt)

            OT = newact()
            if 'attn' in SKIP:
                for kk in range(2):
                    nc.vector.memset(OT[:, kk, :], 0.0)
            for h in (range(NH) if 'attn' not in SKIP else []):
                kc = (h * DH) // 128
                ko = (h * DH) % 128
                attn = []
                for (q0, qn) in QT:
                    ps = ppool.tile([128, NQ], F32, tag="ps", name="ps")
                    nc.tensor.matmul(ps[:qn, :], _r(qk_sl("q", h, slice(q0, q0 + qn))), _r(qk_sl("k", h, slice(None))),
                                     start=True, stop=True)
                    # logits bounded (|x| < 2 for this model's scale); exp directly
                    ex = tmp.tile([128, NQ], BF16, tag="sattn", name="ex", bufs=3)
                    nc.scalar.activation(out=ex[:qn, :], in_=ps[:qn, :], func=ACT.Exp)
                    sm = tmp.tile([128, 1], F32, tag="srow", bufs=3)
                    nc.vector.tensor_reduce(out=sm[:qn, :], in_=ex[:qn, :], op=ALU.add, axis=AX.X)
                    rc = tmp.tile([128, 1], F32, tag="srow", bufs=3)
                    nc.vector.reciprocal(out=rc[:qn, :], in_=sm[:qn, :])
                    nc.vector.tensor_scalar(out=ex[:qn, :], in0=ex[:qn, :], scalar1=rc[:qn, :], scalar2=None,
                                            op0=ALU.mult)
                    attn.append((ex, q0, qn))
                attnT = []
                for (k0, kn) in QT:
                    at = tmp.tile([128, NQ], BF16, tag="sattnT", name="at", bufs=3)
                    for (aw_, q0, qn) in attn:
                        pst = ppool.tile([128, 128], BF16, tag="pst", name="pst", bufs=2)
                        nc.tensor.transpose(pst[:kn, :qn], aw_[:qn, k0:k0 + kn], sb_identb[:qn, :qn])
                        nc.vector.tensor_copy(out=at[:kn, q0:q0 + qn], in_=pst[:kn, :qn])
                    attnT.append((at, k0, kn))
                ps_o = ppool.tile([DH, NQ], F32, tag="pst", name="ps_o", bufs=2)
                for ci, (at, k0, kn) in enumerate(attnT):
                    nc.tensor.matmul(ps_o[:, :], _r(v_nat[ci][:kn, h * DH:(h + 1) * DH]), _r(at[:kn, :]),
                                     start=(ci == 0), stop=(ci == 2))
                nc.vector.tensor_copy(out=OT[ko:ko + DH, kc, :], in_=ps_o[:, :])

            t2 = proj_T(OT, sb_wo, sb_wob)
            x1 = newact()
            for kk in range(2):
                nc.vector.tensor_tensor(out=x1[:, kk, :], in0=sb_tgt[:, kk, :], in1=t2[:, kk, :], op=ALU.add)
            x1n = layer_norm(x1, 0, 1, newact())  # norm2

            # ================= deformable attention =================
            # packed value table: u32 element (pixel) = 2 bf16 channel-lanes;
            # partition p = head p//16, lanes = channels (p//16)*32 + (p%16)*2 + lane
            # one tile per level so gathers can start before the whole table is built
            LVB = [(0, 15002), (15000, 3752), (18750, 952), (19700, 250)]
            LSIZE = [15000, 3750, 950, 247]
            LBASE = [0, 15000, 18750, 19700]
            U32 = mybir.dt.uint32
            vtabs = [single.tile([128, LVB[l][1]], U32, name=f"vt{l}") for l in range(NL)]
            for l in range(NL):
                nc.vector.memset(vtabs[l][:, LSIZE[l]:], 0)
            MCH = 1024
            if 'value' in SKIP:
                for l in range(NL):
                    nc.vector.memset(vtabs[l][:, :], 0)
            def vtab_lane(lv, a, b, lane):
                return vtabs[lv][:, a:b].bitcast(BF16).rearrange("p (a b) -> p a b", b=2)[:, :, lane]
            for lv in ([1, 2, 3, 0] if 'value' not in SKIP else []):
                lsz = LSIZE[lv]
                for mi in range((lsz + MCH - 1) // MCH):
                    s0 = mi * MCH
                    sn = min(MCH, lsz - s0)
                    mt_ = mpool.tile([128, 2, MCH], BF16, tag="mem", name="mt_", bufs=2)
                    mv = mt_[:, :, :sn]
                    nc.sync.dma_start(out=mv,
                                      in_=bass.AP(tensor=memTb, offset=LBASE[lv] + s0,
                                                  ap=[[S, 128], [128 * S, 2], [1, sn]]))
                    for sub in range((sn + 511) // 512):
                        c0 = sub * 512
                        cn = min(512, sn - c0)
                        for lane in range(2):
                            ps = vpool.tile([128, 1024], F32, tag="vwps", name="vps")
                            for kk in range(2):
                                nc.tensor.matmul(ps[:, :cn], sb_wvd[:, kk, lane * 128:(lane + 1) * 128],
                                                 mt_[:, kk, c0:c0 + cn], start=(kk == 0), stop=(kk == 1))
                            nc.scalar.activation(out=vtab_lane(lv, s0 + c0, s0 + c0 + cn, lane), in_=ps[:, :cn],
                                                 func=ACT.Identity, bias=sb_vdb[:, lane, :])

            q2 = newact()
            for kk in range(2):
                nc.vector.tensor_tensor(out=q2[:, kk, :], in0=x1n[:, kk, :], in1=sb_pos[:, kk, :], op=ALU.add)

            zt = single.tile([128, 160], F32, name="zt")
            nc.vector.memset(zt[:, :], 0.0)
            zp = zt[:, :].bitcast(BF16)
            nc.sync.dma_start(out=bass.AP(tensor=wdram, offset=0, ap=[[1280, 128], [1, 1280]]),
                              in_=bass.AP(tensor=zp.tensor, offset=zp.offset,
                                          ap=[list(zp.ap[0]), [0, 4], [1, 320]]))
            I16 = mybir.dt.int16
            idxs = single.tile([128, 1280], I16)
            ji = [single.tile([128, 304], I16, name=f"ji{m}") for m in range(4)]
            for m in range(4):
                nc.vector.memset(ji[m][:, :], 0)
            zu = zt[:, :].bitcast(I16)
            nc.sync.dma_start(out=bass.AP(tensor=jdram, offset=0, ap=[[1280, 128], [1, 1280]]),
                              in_=bass.AP(tensor=zu.tensor, offset=zu.offset,
                                          ap=[list(zu.ap[0]), [0, 4], [1, 320]]))
            if 'samp' in SKIP:
                pass
            for ti, (q0, qn) in (list(enumerate(QT)) if 'samp' not in SKIP else []):
                rt = tmp.tile([128, 8], F32, tag="refs", name="rt", bufs=1)
                nc.sync.dma_start(out=rt[:qn, :], in_=refs[q0:q0 + qn, :])
                ps_off = ppool.tile([128, D], F32, tag="ps")
                for kk in range(2):
                    nc.tensor.matmul(ps_off[:qn, :], _r(q2[:, kk, q0:q0 + qn]), _r(sb_wso[:, kk, :]),
                                     start=(kk == 0), stop=(kk == 1))
                off = tmp.tile([128, D], F32, tag="off", name="off", bufs=1)
                nc.vector.tensor_tensor(out=off[:qn, :], in0=ps_off[:qn, :], in1=SOBr[:qn, :], op=ALU.add)
                ps_aw = ppool.tile([128, 128], F32, tag="pst", name="ps_aw", bufs=2)
                for kk in range(2):
                    nc.tensor.matmul(ps_aw[:qn, :], _r(q2[:, kk, q0:q0 + qn]), _r(sb_waw[:, kk, :]),
                                     start=(kk == 0), stop=(kk == 1))
                awl = tmp.tile([128, 128], F32, tag="aw")
                nc.vector.tensor_tensor(out=awl[:qn, :], in0=ps_aw[:qn, :], in1=AWBr[:qn, :], op=ALU.add)
                # aw logits bounded (|x| < 3 for this model's scale); exp directly
                nc.scalar.activation(out=awl[:qn, :], in_=awl[:qn, :], func=ACT.Exp)
                sm = tmp.tile([128, NH], F32, tag="awrow", bufs=3)
                nc.vector.tensor_reduce(out=sm[:qn, :], in_=awl[:qn, :].rearrange("p (h g) -> p h g", g=16),
                                        op=ALU.add, axis=AX.X)
                rc = tmp.tile([128, NH], F32, tag="awrow", bufs=3)
                nc.vector.reciprocal(out=rc[:qn, :], in_=sm[:qn, :])
                rca = rc[:qn, :]
                aw = tmp.tile([128, 128], F32, tag="aw")
                nc.vector.tensor_tensor(out=aw[:qn, :].rearrange("p (h g) -> p h g", g=16),
                                        in0=awl[:qn, :].rearrange("p (h g) -> p h g", g=16),
                                        in1=bass.AP(tensor=rca.tensor, offset=rca.offset,
                                                    ap=[list(rca.ap[0]), list(rca.ap[1]), [0, 16]]),
                                        op=ALU.mult)

                J = tmp.tile([128, 512], F32, tag="J", name="J", bufs=1)
                Wt = tmp.tile([128, 512], BF16, tag="Wt", name="Wt", bufs=1)
                ra = rt[:qn, :]
                p_ = tmp.tile([128, 256], F32, tag="pxy", name="p_", bufs=2)
                for xy in range(2):
                    refb = bass.AP(tensor=ra.tensor, offset=ra.offset + xy,
                                   ap=[list(ra.ap[0]), [0, NH], [2, NL], [0, NP]])
                    nc.vector.tensor_tensor(
                        out=p_[:qn, xy * 128:(xy + 1) * 128].rearrange("p (h l m) -> p h l m", h=NH, l=NL),
                        in0=refb,
                        in1=SCLr[:qn, xy * 128:(xy + 1) * 128].rearrange("p (h l m) -> p h l m", h=NH, l=NL),
                        op=ALU.mult)
                nc.vector.tensor_tensor(out=p_[:qn, :], in0=p_[:qn, :], in1=off[:qn, :], op=ALU.add)
                # shift +64 so trunc == floor; consts pre-shifted on host
                nc.vector.tensor_scalar(out=p_[:qn, :], in0=p_[:qn, :], scalar1=63.5, scalar2=None, op0=ALU.add)
                xi = tmp.tile([128, 256], mybir.dt.int32, tag="scr", name="xi", bufs=2)
                nc.vector.tensor_copy(out=xi[:qn, :], in_=p_[:qn, :])
                x0 = tmp.tile([128, 256], F32, tag="x0m", name="x0", bufs=1)
                nc.vector.tensor_copy(out=x0[:qn, :], in_=xi[:qn, :])
                # cast may trunc or round-to-nearest; fix up to floor either way
                gt_ = tmp.tile([128, 256], F32, tag="scr", name="gt_", bufs=2)
                nc.vector.tensor_tensor(out=gt_[:qn, :], in0=x0[:qn, :], in1=p_[:qn, :], op=ALU.is_gt)
                nc.vector.tensor_tensor(out=x0[:qn, :], in0=x0[:qn, :], in1=gt_[:qn, :], op=ALU.subtract)
                nc.vector.tensor_scalar(out=x0[:qn, :], in0=x0[:qn, :], scalar1=64.0, scalar2=None, op0=ALU.max)
                nc.vector.tensor_tensor(out=x0[:qn, :], in0=x0[:qn, :], in1=CM2r[:qn, :], op=ALU.min)
                w0 = tmp.tile([128, 256], F32, tag="w0m", name="w0", bufs=1)
                w1_ = tmp.tile([128, 256], F32, tag="w1m", name="w1_", bufs=1)
                dt_ = tmp.tile([128, 256], F32, tag="scr", name="dt_", bufs=2)
                nc.vector.tensor_tensor(out=dt_[:qn, :], in0=p_[:qn, :], in1=x0[:qn, :], op=ALU.subtract)
                ab0 = tmp.tile([128, 256], F32, tag="scr", name="ab0", bufs=2)
                nc.scalar.activation(out=ab0[:qn, :], in_=dt_[:qn, :], func=ACT.Abs)
                nc.vector.tensor_scalar(out=ab0[:qn, :], in0=ab0[:qn, :], scalar1=-1.0, scalar2=1.0,
                                        op0=ALU.mult, op1=ALU.add)
                nc.vector.tensor_scalar(out=w0[:qn, :], in0=ab0[:qn, :], scalar1=0.0, scalar2=None, op0=ALU.max)
                nc.vector.tensor_scalar(out=dt_[:qn, :], in0=dt_[:qn, :], scalar1=-1.0, scalar2=None, op0=ALU.add)
                nc.scalar.activation(out=ab0[:qn, :], in_=dt_[:qn, :], func=ACT.Abs)
                nc.vector.tensor_scalar(out=ab0[:qn, :], in0=ab0[:qn, :], scalar1=-1.0, scalar2=1.0,
                                        op0=ALU.mult, op1=ALU.add)
                nc.vector.tensor_scalar(out=w1_[:qn, :], in0=ab0[:qn, :], scalar1=0.0, scalar2=None, op0=ALU.max)
                (xx0, wx0, wx1) = (x0[:, :128], w0[:, :128], w1_[:, :128])
                (yy0, wy0, wy1) = (x0[:, 128:], w0[:, 128:], w1_[:, 128:])
                jb = tmp.tile([128, 128], F32, tag="jb", name="jb", bufs=1)
                nc.vector.tensor_tensor(out=jb[:qn, :], in0=yy0[:qn, :], in1=WLr[:qn, :], op=ALU.mult)
                nc.vector.tensor_tensor(out=jb[:qn, :], in0=jb[:qn, :], in1=xx0[:qn, :], op=ALU.add)
                nc.vector.tensor_tensor(out=jb[:qn, :], in0=jb[:qn, :], in1=LSIr[:qn, :], op=ALU.add)
                # J col = h*64 + p*16 + c*8 + l*2 + pix; jb cols are (h:16, l:4, p:1)
                ja = J[:qn, :]
                def jview(c, x):
                    return bass.AP(tensor=ja.tensor, offset=ja.offset + c * 8 + x,
                                   ap=[list(ja.ap[0]), [64, 8], [2, 4], [16, 4]])
                nc.vector.tensor_copy(out=jview(0, 0), in_=jb[:qn, :])
                nc.vector.tensor_tensor(out=jview(1, 0), in0=jb[:qn, :], in1=WLr[:qn, :], op=ALU.add)
                nc.vector.tensor_scalar(out=jview(0, 1), in0=jview(0, 0), scalar1=1.0, scalar2=None,
                                        op0=ALU.add)
                nc.vector.tensor_scalar(out=jview(1, 1), in0=jview(1, 0), scalar1=1.0, scalar2=None,
                                        op0=ALU.add)
                t_c = []
                for c, wyc in ((0, wy0), (1, wy1)):
                    tc_ = tmp.tile([128, 128], F32, tag=f"tc{c}", name="tc_", bufs=1)
                    nc.vector.tensor_tensor(out=tc_[:qn, :], in0=aw[:qn, :], in1=wyc[:qn, :], op=ALU.mult)
                    t_c.append(tc_)
                Wv4 = Wt[:qn, :].rearrange("p (f c s) -> p f c s", c=2, s=2)
                for c in range(2):
                    for sp, wxv in ((0, wx0), (1, wx1)):
                        nc.vector.tensor_tensor(out=Wv4[:, :, c, sp], in0=t_c[c][:qn, :], in1=wxv[:qn, :],
                                                op=ALU.mult)
                wa = Wt[:qn, :]
                nc.sync.dma_start(
                    out=bass.AP(tensor=wdram, offset=q0 * 16,
                                ap=[[16, qn], [WQPAD * 64, NH], [5120, 4], [4, 4], [2, 2], [1, 2]]),
                    in_=bass.AP(tensor=wa.tensor, offset=wa.offset,
                                ap=[list(wa.ap[0]), [64, NH], [16, 4], [4, 4], [2, 2], [1, 2]]))
                for m in range(4):
                    pst = ppool.tile([128, 128], F32, tag="pst", name="pst", bufs=2)
                    nc.tensor.transpose(pst[:, :qn], J[:qn, m * 128:(m + 1) * 128], sb_ident[:qn, :qn])
                    pv = pst[:, :qn].rearrange("p (a b) -> p a b", b=2)
                    nc.vector.tensor_copy(out=ji[m][:, q0 // 2:q0 // 2 + qn // 2], in_=pv[:, :, 0])
                    nc.vector.tensor_copy(out=ji[m][:, 152 + q0 // 2:152 + q0 // 2 + qn // 2], in_=pv[:, :, 1])

            for m in range(4):
                # ji[m] rows = (h2, p, c, l, pix); jdram elem = P*1280 + lv*320 + qh*2 + pix,
                # P = (2m+h2)*16 + qm2*8 + p*2 + c; per-pix slices collapse to <=3 dims
                jm = ji[m][:, :]
                pp_ = jm.ap[0][0]
                for qm2 in range(2):
                    for pix in range(2):
                        eng = nc.sync if (m % 2 == 0) else nc.scalar
                        eng.dma_start(
                            out=bass.AP(tensor=jdram,
                                        offset=m * 2 * 20480 + qm2 * 10240 + pix * 160,
                                        ap=[[20480, 2], [320, 32], [1, 152]]),
                            in_=bass.AP(tensor=jm.tensor,
                                        offset=jm.offset + pix * pp_ + qm2 * 152,
                                        ap=[[2 * pp_, 64], [1, 152]]))
            iap = idxs[:, :]
            nc.sync.dma_start(
                out=bass.AP(tensor=iap.tensor, offset=iap.offset, ap=[list(iap.ap[0]), [1, 1280]]),
                in_=bass.AP(tensor=jdram, offset=0, ap=[[1, 163840]]))

            wsb = single.tile([NH * NSEL, SUBW], BF16)
            nc.sync.dma_start(out=wsb[:, :],
                              in_=bass.AP(tensor=wdram, offset=0, ap=[[WQPAD * 64, NH], [SUBW, NSEL], [1, SUBW]]))

            ODl = [actp.tile([128, 320, 2], BF16, name=f"OD_{l}") for l in range(NL)]
            # per level: 2 ap_gather calls (qh 0..96, 96..160); idx stream per 16-group:
            # i = qh*32 + pix*16 + (qm2*8 + p*2 + c); gathered u32 = 2 bf16 ch-lanes
            # per level: 2 ap_gather calls (pix=0: x0 pixels, pix=1: x0+1);
            # stream i = qh*16 + qm2*8 + p*2 + c per 16-group; u32 = 2 bf16 ch-lanes
            for lv in ([1, 2, 3, 0] if 'gather' not in SKIP else []):
                b0, blen = LVB[lv]
                gts = {}
                for pix in range(2):
                    gt = gpool.tile([128, 2560], mybir.dt.uint32, tag="gather", name="gt", bufs=2)
                    nc.gpsimd.ap_gather(out_ap=gt[:, :], in_ap=vtabs[lv][:, :],
                                        idxs_ap=idxs[:, lv * 320 + pix * 160:lv * 320 + pix * 160 + 160],
                                        channels=128, num_elems=blen, d=1, num_idxs=2560)
                    gts[pix] = gt
                for gi in range(5):
                    sl0 = gi * 32
                    psw = vpool.tile([128, 1024], F32, tag="vwps", name="psw")
                    for half in range(2):
                        el0 = lv * 5120 + (sl0 + half * 16) * 32
                        sub = el0 // SUBW
                        eoff = el0 % SUBW
                        nc.tensor.matmul(psw[:, half * 512:(half + 1) * 512],
                                         sb_sel[:, sub * 128:(sub + 1) * 128],
                                         wsb[:, eoff:eoff + 512], start=True, stop=True)
                    prod = mpool.tile([128, 2048], BF16, tag="prod", name="prod", bufs=2)
                    pr = prod[:, :]
                    pa = psw[:, :]
                    for pix in range(2):
                        gh = gts[pix][:, :].bitcast(BF16)  # (i, lane)
                        # iteration (qh32, qm2, p, c, lane)
                        ga = bass.AP(tensor=gh.tensor, offset=gh.offset + sl0 * 32,
                                     ap=[list(gh.ap[0]), [32, 32], [16, 2], [4, 4], [2, 2], [1, 2]])
                        pb = bass.AP(tensor=pa.tensor, offset=pa.offset + pix,
                                     ap=[list(pa.ap[0]), [32, 32], [16, 2], [4, 4], [2, 2], [0, 2]])
                        # prod layout: qh*64 + qm2*32 + lane*16 + p*4 + c*2 + pix
                        pw = bass.AP(tensor=pr.tensor, offset=pr.offset + pix,
                                     ap=[list(pr.ap[0]), [64, 32], [32, 2], [4, 4], [2, 2], [16, 2]])
                        nc.vector.tensor_tensor(out=pw, in0=ga, in1=pb, op=ALU.mult)
                    # reduce over contiguous (p,c,pix)=16 keeping (qh, qm2, lane)
                    ra = bass.AP(tensor=pr.tensor, offset=pr.offset,
                                 ap=[list(pr.ap[0]), [64, 32], [32, 2], [16, 2], [1, 16]])
                    ov = ODl[lv][:, sl0 * 2:sl0 * 2 + 64, :]
                    ow = bass.AP(tensor=ov.tensor, offset=ov.offset,
                                 ap=[list(ov.ap[0]), [4, 32], [2, 2], [1, 2]])
                    with nc.allow_low_precision(reason="16-term bf16 reduce, tol 2e-2"):
                        nc.vector.tensor_reduce(out=ow, in_=ra, op=ALU.add, axis=AX.X)
            if 'gather' in SKIP:
                for lv in range(NL):
                    nc.vector.memset(ODl[lv][:, :, :], 0.0)
            t2d = newact()
            for mt in range(2):
                ps = ppool.tile([128, NQ], F32, tag="ps")
                for ci, (lane, lv) in enumerate([(l_, v_) for l_ in range(2) for v_ in range(NL)]):
                    nc.tensor.matmul(ps[:, :], sb_wod[:, lane, mt * 128:(mt + 1) * 128],
                                     ODl[lv][:, :NQ, lane], start=(ci == 0), stop=(ci == 7))
                nc.vector.tensor_scalar(out=t2d[:, mt, :], in0=ps[:, :], scalar1=sb_wodb[:, mt, :],
                                        scalar2=None, op0=ALU.add)
            x2 = newact()
            for kk in range(2):
                nc.vector.tensor_tensor(out=x2[:, kk, :], in0=x1n[:, kk, :], in1=t2d[:, kk, :], op=ALU.add)
            x2n = layer_norm(x2, 2, 3, newact())  # norm1

            # ================= FFN =================
            h1 = actp.tile([128, 8, NQ], BF16)
            if 'ffn' in SKIP:
                for mt in range(8):
                    nc.vector.memset(h1[:, mt, :], 0.0)
            for mt in (range(8) if 'ffn' not in SKIP else []):
                ps = ppool.tile([128, NQ], F32, tag="ps")
                wt1 = mpool.tile([128, 2, 128], BF16, tag="w1s", name="wt1", bufs=2)
                nc.sync.dma_start(out=wt1[:, :, :],
                                  in_=bass.AP(tensor=w1T, offset=mt * 128,
                                              ap=[[DFF, 128], [128 * DFF, 2], [1, 128]]))
                for kk in range(2):
                    nc.tensor.matmul(ps[:, :], wt1[:, kk, :], x2n[:, kk, :],
                                     start=(kk == 0), stop=(kk == 1))
                nc.scalar.activation(out=h1[:, mt, :], in_=ps[:, :], func=ACT.Relu, bias=sb_b1[:, mt, :])
            t2f = newact()
            for mt in range(2):
                ps = ppool.tile([128, NQ], F32, tag="ps")
                wt2 = mpool.tile([128, 8, 128], BF16, tag="w2s8", name="w2s", bufs=2)
                nc.sync.dma_start(out=wt2[:, :, :],
                                  in_=bass.AP(tensor=w2T, offset=mt * 128,
                                              ap=[[D, 128], [128 * D, 8], [1, 128]]))
                for kk in range(8):
                    nc.tensor.matmul(ps[:, :], wt2[:, kk, :], h1[:, kk, :],
                                     start=(kk == 0), stop=(kk == 7))
                nc.vector.tensor_scalar(out=t2f[:, mt, :], in0=ps[:, :], scalar1=sb_b2[:, mt, :],
                                        scalar2=None, op0=ALU.add)
            x3 = newact()
            for kk in range(2):
                nc.vector.tensor_tensor(out=x3[:, kk, :], in0=x2n[:, kk, :], in1=t2f[:, kk, :], op=ALU.add)
            x3n = layer_norm(x3, 4, 5, actp.tile([128, 2, NQ], F32, name="actsf"))  # norm3
            for kk in range(2):
                nc.sync.dma_start(out=outT[kk * 128:(kk + 1) * 128, :], in_=x3n[:, kk, :])

    nc.compile()
    return nc


def _perm_so():
    # samp_off_w rows are (h, l, p, xy); reorder to (xy, h, l, p)
    return np.array([((h * NL + l) * NP + p) * 2 + xy
                     for xy in range(2) for h in range(NH) for l in range(NL) for p in range(NP)])


def _host_prep(inputs):
    f = lambda x: np.ascontiguousarray(np.asarray(x, dtype=np.float32))
    in_w = f(inputs["in_proj_w"]); in_b = f(inputs["in_proj_b"])
    qw, kw, vw = in_w[:D], in_w[D:2 * D], in_w[2 * D:]
    qb_, kb_, vb_ = in_b[:D], in_b[D:2 * D], in_b[2 * D:]
    sc = 1.0 / np.sqrt(DH)
    perm = np.array([(p // 16) * 32 + (p % 16) * 2 + lane
                     for lane in range(2) for p in range(128)])
    shared = {
        "qb": (qb_ * sc)[:, None], "kb": kb_[:, None],
        "wob": f(inputs["out_proj_b"])[:, None],
        "vdb": f(inputs["value_b"])[perm][:, None],
        "wodb": f(inputs["outp_b"])[:, None],
        "b1": f(inputs["lin1_b"])[:, None],
        "b2": f(inputs["lin2_b"])[:, None],
        "ln_gb": np.stack([f(inputs["norm2_g"]), f(inputs["norm2_b"]),
                           f(inputs["norm1_g"]), f(inputs["norm1_b"]),
                           f(inputs["norm3_g"]), f(inputs["norm3_b"])], axis=1),
        "ident_in": np.eye(128, dtype=np.float32),
    }
    Wv_ = SPATIAL[:, 1].astype(np.float32); Hv_ = SPATIAL[:, 0].astype(np.float32)
    row = lambda vals: np.tile(np.repeat(vals, NP), NH)
    lsi_adj = -64.0 * Wv_ - 64.0  # level-local indices
    pad128 = np.zeros(128, np.float32)
    shared["consts"] = np.stack([
        np.concatenate([row(Wv_), row(Hv_)]),
        np.concatenate([row(Wv_ + 62), row(Hv_ + 62)]),
        np.concatenate([row(lsi_adj), pad128]),
        np.concatenate([f(inputs["attn_wt_b"]), pad128])]).astype(np.float32)
    shared["consts2"] = np.stack([f(inputs["samp_off_b"])[_perm_so()], vb_]).astype(np.float32)
    sel = np.zeros((NH * NSEL, NSEL * 128), dtype=np.float32)
    for s_ in range(NSEL):
        for p in range(128):
            sel[(p // 16) * NSEL + s_, s_ * 128 + p] = 1.0
    shared = {k: np.ascontiguousarray(np.asarray(v, np.float32)) for k, v in shared.items()}
    import ml_dtypes
    bf = lambda x: np.ascontiguousarray(np.asarray(x, np.float32).astype(ml_dtypes.bfloat16))
    shared["sel_in"] = bf(sel)
    shared["wvdTb"] = bf(f(inputs["value_w"])[perm].T)
    shared["wodT"] = bf(f(inputs["outp_w"]).T[perm])
    shared["w2T"] = bf(f(inputs["lin2_w"]).T)
    shared["wqT"] = bf((qw * sc).T); shared["wkT"] = bf(kw.T); shared["wvT"] = bf(vw.T)
    shared["woT"] = bf(f(inputs["out_proj_w"]).T)
    shared["wsoT"] = bf(f(inputs["samp_off_w"])[_perm_so()].T)
    shared["wawT"] = bf(f(inputs["attn_wt_w"]).T)
    shared["w1T"] = bf(f(inputs["lin1_w"]).T)
    shared["identb_in"] = bf(np.eye(128, dtype=np.float32))
    per_core = []
    for b in range(BS):
        m = dict(shared)
        m["tgtT"] = bf(f(inputs["tgt"][b]).T)
        m["posT"] = bf(f(inputs["tgt_query_pos"][b]).T)
        m["memTb"] = np.ascontiguousarray(f(inputs["memory"][b]).T.astype(ml_dtypes.bfloat16))
        m["refs"] = np.ascontiguousarray(f(inputs["tgt_reference_points"][b]).reshape(NQ, 8))
        per_core.append(m)
    return per_core


def kernel(**inputs) -> np.ndarray:
    if "nc" not in _cache:
        _cache["nc"] = build_bass()
    nc = _cache["nc"]
    in_maps = _host_prep(inputs)
    res = run_bass_kernel_spmd(nc, in_maps, core_ids=list(range(BS)))
    out = np.stack([np.ascontiguousarray(r["outT"].T) for r in res.results])
    return out.astype(np.float32)



# revision 51
# speedup vs baseline: 1.8960x; 1.0858x over previous
import numpy as np

import concourse.bass as bass
from concourse import bacc
import concourse.mybir as mybir
import concourse.tile as tile
from concourse.bass_utils import run_bass_kernel_spmd

# ---- problem constants (hardcoded) ----
D = 256; NH = 8; NL = 4; NP = 4; DFF = 1024; BS = 8; NQ = 300
DH = D // NH  # 32
EPS = 1e-5
SPATIAL = np.array([[100, 150], [50, 75], [25, 38], [13, 19]], dtype=np.int64)
SIZES = (SPATIAL[:, 0] * SPATIAL[:, 1])
S = int(SIZES.sum())  # 19947
LSI = np.concatenate([[0], np.cumsum(SIZES)[:-1]]).astype(np.int64)
ST = S + 3  # table padded (pair windows read j, j+1; L3 slice needs +2); even

F32 = mybir.dt.float32
F32R = mybir.dt.float32r
BF16 = mybir.dt.bfloat16
U16 = mybir.dt.uint16
ALU = mybir.AluOpType
ACT = mybir.ActivationFunctionType
AX = mybir.AxisListType

QT = [(0, 128), (128, 128), (256, 44)]
GCH = [(i * 32, 32) for i in range(9)] + [(288, 16)]
WQPAD = 320
NSEL = 8
SUBW = 2560

_cache = {}
import os
SKIP = set(os.environ.get('KSKIP','').split(','))


USE_F32R = False


def _r(ap):
    return ap.bitcast(F32R) if USE_F32R else ap


def build_bass():
    nc = bacc.Bacc("TRN2", target_bir_lowering=False)
    tpT = nc.dram_tensor("tpT", [D, 2 * NQ], BF16, kind="ExternalInput")
    memTb = nc.dram_tensor("memTb", [D, S], BF16, kind="ExternalInput")
    refs = nc.dram_tensor("refs", [NQ, 8], F32, kind="ExternalInput")
    wbig = nc.dram_tensor("wbig", [D, 2944], BF16, kind="ExternalInput")
    w2T = nc.dram_tensor("w2T", [DFF, D], BF16, kind="ExternalInput")
    fsm = nc.dram_tensor("fsm", [D, 20], F32, kind="ExternalInput")
    consts = nc.dram_tensor("consts", [4, 256], F32, kind="ExternalInput")
    consts2 = nc.dram_tensor("consts2", [2, D], F32, kind="ExternalInput")
    ident_in = nc.dram_tensor("ident_in", [128, 128], F32, kind="ExternalInput")
    identb_in = nc.dram_tensor("identb_in", [128, 128], BF16, kind="ExternalInput")
    sel_in = nc.dram_tensor("sel_in", [NH * NSEL, 128 * NSEL], BF16, kind="ExternalInput")
    outT = nc.dram_tensor("outT", [D, NQ], F32, kind="ExternalOutput")
    wdram = nc.dram_tensor("wdram", [NH, WQPAD * 64], BF16)
    jdram = nc.dram_tensor("jdram", [163840], mybir.dt.int16)

    with tile.TileContext(nc) as tc:
        import contextlib
        ctx = contextlib.ExitStack()
        with ctx:
            single = ctx.enter_context(tc.tile_pool(name="single", bufs=1))
            actp = ctx.enter_context(tc.tile_pool(name="actp", bufs=1))
            tmp = ctx.enter_context(tc.tile_pool(name="tmp", bufs=2))
            mpool = ctx.enter_context(tc.tile_pool(name="mpool", bufs=3))
            gpool = ctx.enter_context(tc.tile_pool(name="gpool", bufs=2))
            ppool = ctx.enter_context(tc.tile_pool(name="ppool", bufs=2, space="PSUM"))
            vpool = ctx.enter_context(tc.tile_pool(name="vpool", bufs=2, space="PSUM"))
            

            # stacked loader: DRAM [K*128, X] -> SBUF [128, K, X]
            def loadS(dram, rows, cols, pool=single, dt=F32):
                k = rows // 128
                nm = "w_" + dram.name
                t = pool.tile([128, k, cols], dt, name=nm, tag=nm)
                for i in range(k):
                    nc.sync.dma_start(out=t[:, i, :], in_=dram[i * 128:(i + 1) * 128, :])
                return t

            def loadC(dram, rows, cols, pool=single, dt=F32):  # small col tiles [rows<=128, cols]
                nm = "w_" + dram.name
                t = pool.tile([rows, cols], dt, name=nm, tag=nm)
                nc.sync.dma_start(out=t[:, :], in_=dram[:, :])
                return t

            sb_wbig = single.tile([128, 2, 2944], BF16, name="w_big", tag="w_big")
            nc.sync.dma_start(out=sb_wbig[:, :, :],
                              in_=bass.AP(tensor=wbig, offset=0,
                                          ap=[[2944, 128], [128 * 2944, 2], [1, 2944]]))
            sb_wq = sb_wbig[:, :, 0:256]; sb_wk = sb_wbig[:, :, 256:512]
            sb_wv = sb_wbig[:, :, 512:768]; sb_wo = sb_wbig[:, :, 768:1024]
            sb_wso = sb_wbig[:, :, 1024:1280]; sb_waw = sb_wbig[:, :, 1280:1408]
            sb_wvd = sb_wbig[:, :, 1408:1664]; sb_wod = sb_wbig[:, :, 1664:1920]
            sb_w1 = sb_wbig[:, :, 1920:2944]
            sb_w2 = single.tile([128, 8, D], BF16, name="w_w2", tag="w_w2")
            nc.sync.dma_start(out=sb_w2[:, :, :],
                              in_=bass.AP(tensor=w2T, offset=0,
                                          ap=[[D, 128], [128 * D, 8], [1, D]]))
            sb_fsm = single.tile([128, 2, 20], F32, name="w_fsm", tag="w_fsm")
            nc.sync.dma_start(out=sb_fsm[:, :, :],
                              in_=bass.AP(tensor=fsm, offset=0,
                                          ap=[[20, 128], [128 * 20, 2], [1, 20]]))
            sb_qb = sb_fsm[:, :, 0:1]; sb_kb = sb_fsm[:, :, 1:2]
            sb_wob = sb_fsm[:, :, 2:3]; sb_vdb = sb_fsm[:, :, 3:4]
            sb_wodb = sb_fsm[:, :, 4:5]; sb_b2 = sb_fsm[:, :, 5:6]
            sb_lngb = sb_fsm[:, :, 6:12]
            sb_ident = loadC(ident_in, 128, 128)
            sb_identb = loadC(identb_in, 128, 128, dt=BF16)
            sb_sel = loadC(sel_in, NH * NSEL, 128 * NSEL, dt=BF16)

            sb_consts = single.tile([128, 4, 256], F32)
            cap = consts[:, :]
            nc.sync.dma_start(out=sb_consts[:, :, :],
                              in_=bass.AP(tensor=cap.tensor, offset=cap.offset, ap=[[0, 128], [256, 4], [1, 256]]))
            SCLr = sb_consts[:, 0, :]; CM2r = sb_consts[:, 1, :]
            WLr = sb_consts[:, 0, :128]; LSIr = sb_consts[:, 2, :128]
            AWBr = sb_consts[:, 3, :128]
            sb_consts2 = single.tile([128, 2, D], F32)
            c2 = consts2[:, :]
            nc.sync.dma_start(out=sb_consts2[:, :, :],
                              in_=bass.AP(tensor=c2.tensor, offset=c2.offset, ap=[[0, 128], [D, 2], [1, D]]))
            SOBr = sb_consts2[:, 0, :]; VBSr = sb_consts2[:, 1, :]

            ones_col = single.tile([128, 1], BF16)
            nc.vector.memset(ones_col[:, :], 1.0)
            ones_row = single.tile([1, 128], BF16)
            nc.vector.memset(ones_row[:, :], 1.0)

            # activations stacked [128, 2, NQ]
            def newact():
                return actp.tile([128, 2, NQ], BF16, tag="acts", name="acts", bufs=3)

            sb_tp = actp.tile([128, 2, 2 * NQ], BF16)
            nc.sync.dma_start(out=sb_tp[:, :, :],
                              in_=bass.AP(tensor=tpT, offset=0,
                                          ap=[[2 * NQ, 128], [128 * 2 * NQ, 2], [1, 2 * NQ]]))
            sb_tgt = sb_tp[:, :, 0:NQ]
            sb_pos = sb_tp[:, :, NQ:2 * NQ]

            def layer_norm(xT, gi, bi, out_t):
                ps_s = ppool.tile([1, NQ], F32, tag="ps", name="ps_s")
                ps_q = ppool.tile([1, NQ], F32, tag="ps", name="ps_q")
                for kk in range(2):
                    nc.tensor.matmul(ps_s[:, :], ones_col[:, :], xT[:, kk, :], start=(kk == 0), stop=(kk == 1))
                for kk in range(2):
                    sq = tmp.tile([128, NQ], BF16, tag="lnsq", name="sq", bufs=1)
                    nc.vector.tensor_tensor(out=sq[:, :], in0=xT[:, kk, :], in1=xT[:, kk, :], op=ALU.mult)
                    nc.tensor.matmul(ps_q[:, :], ones_col[:, :], sq[:, :], start=(kk == 0), stop=(kk == 1))
                mean = tmp.tile([1, NQ], F32, tag="lnrow", bufs=4)
                nc.vector.tensor_scalar(out=mean[:, :], in0=ps_s[:, :], scalar1=1.0 / D, scalar2=None, op0=ALU.mult)
                var = tmp.tile([1, NQ], F32, tag="lnrow", bufs=4)
                nc.vector.tensor_scalar(out=var[:, :], in0=ps_q[:, :], scalar1=1.0 / D, scalar2=None, op0=ALU.mult)
                m2 = tmp.tile([1, NQ], F32, tag="lnrow", bufs=4)
                nc.vector.tensor_tensor(out=m2[:, :], in0=mean[:, :], in1=mean[:, :], op=ALU.mult)
                nc.vector.tensor_tensor(out=var[:, :], in0=var[:, :], in1=m2[:, :], op=ALU.subtract)
                nc.vector.tensor_scalar(out=var[:, :], in0=var[:, :], scalar1=EPS, scalar2=None, op0=ALU.add)
                nc.scalar.sqrt(out=var[:, :], in_=var[:, :])
                rstd = tmp.tile([1, NQ], BF16, tag="lnrowb", bufs=2)
                with nc.allow_low_precision(reason="bf16 rstd, tol 2e-2"):
                    nc.vector.reciprocal(out=rstd[:, :], in_=var[:, :])
                nmr = tmp.tile([1, NQ], BF16, tag="lnrowb", bufs=2)
                nc.vector.tensor_tensor(out=nmr[:, :], in0=mean[:, :], in1=rstd[:, :], op=ALU.mult)
                nc.vector.tensor_scalar(out=nmr[:, :], in0=nmr[:, :], scalar1=-1.0, scalar2=None, op0=ALU.mult)
                ps_r = ppool.tile([128, NQ], F32, tag="ps", name="ps_r")
                ps_m = ppool.tile([128, NQ], F32, tag="ps", name="ps_m")
                nc.tensor.matmul(ps_r[:, :], ones_row[:, :], rstd[:, :], start=True, stop=True)
                nc.tensor.matmul(ps_m[:, :], ones_row[:, :], nmr[:, :], start=True, stop=True)
                for kk in range(2):
                    sl = slice(kk * 128, (kk + 1) * 128)
                    t1 = tmp.tile([128, NQ], F32, tag="lnt", name="t1", bufs=1)
                    nc.vector.tensor_tensor(out=t1[:, :], in0=xT[:, kk, :], in1=ps_r[:, :], op=ALU.mult)
                    nc.vector.tensor_tensor(out=t1[:, :], in0=t1[:, :], in1=ps_m[:, :], op=ALU.add)
                    nc.vector.tensor_scalar(out=out_t[:, kk, :], in0=t1[:, :],
                                            scalar1=sb_lngb[:, kk, gi:gi + 1], scalar2=sb_lngb[:, kk, bi:bi + 1],
                                            op0=ALU.mult, op1=ALU.add)
                return out_t

            # ================= self attention =================
            A = newact()
            for kk in range(2):
                nc.vector.tensor_tensor(out=A[:, kk, :], in0=sb_tgt[:, kk, :], in1=sb_pos[:, kk, :], op=ALU.add)

            def proj_T(src, w, bias_col, dst=None):
                if dst is None:
                    dst = newact()
                for mt in range(2):
                    ps = ppool.tile([128, NQ], F32, tag="ps")
                    for kk in range(2):
                        nc.tensor.matmul(ps[:, :], _r(w[:, kk, mt * 128:(mt + 1) * 128]),
                                         _r(src[:, kk, :]), start=(kk == 0), stop=(kk == 1))
                    if bias_col is not None:
                        nc.vector.tensor_scalar(out=dst[:, mt, :], in0=ps[:, :], scalar1=bias_col[:, mt, :],
                                                scalar2=None, op0=ALU.add)
                    else:
                        nc.vector.tensor_copy(out=dst[:, mt, :], in_=ps[:, :])
                return dst

            # packed q/k: 3 heads per tile at bases 0/32/64
            qk3 = {"q": [actp.tile([96, NQ], BF16, name=f"q3_{i}") for i in range(3)],
                   "k": [actp.tile([96, NQ], BF16, name=f"k3_{i}") for i in range(3)]}
            def qk_sl(which, h, cols):
                return qk3[which][h // 3][(h % 3) * DH:(h % 3) * DH + DH, cols]
            for w_, b_, which in ((sb_wq, sb_qb, "q"), (sb_wk, sb_kb, "k")):
                for mt in range(2):
                    ps = ppool.tile([128, NQ], F32, tag="ps", name="ps")
                    for kk in range(2):
                        nc.tensor.matmul(ps[:, :], _r(w_[:, kk, mt * 128:(mt + 1) * 128]),
                                         _r(A[:, kk, :]), start=(kk == 0), stop=(kk == 1))
                    for hh in range(4):
                        h = mt * 4 + hh
                        nc.scalar.activation(out=qk_sl(which, h, slice(None)),
                                             in_=ps[hh * DH:(hh + 1) * DH, :],
                                             func=ACT.Identity, bias=b_[hh * DH:(hh + 1) * DH, mt, :])
            v_nat = []
            for (q0, qn) in QT:
                ps = ppool.tile([128, D], F32, tag="ps")
                for kk in range(2):
                    nc.tensor.matmul(ps[:qn, :], _r(sb_tgt[:, kk, q0:q0 + qn]), _r(sb_wv[:, kk, :]),
                                     start=(kk == 0), stop=(kk == 1))
                vt = actp.tile([128, D], BF16, tag="vnat", name="vt", bufs=3)
                nc.vector.tensor_tensor(out=vt[:qn, :], in0=ps[:qn, :], in1=VBSr[:qn, :], op=ALU.add)
                v_nat.append(vt)

            OT = newact()
            if 'attn' in SKIP:
                for kk in range(2):
                    nc.vector.memset(OT[:, kk, :], 0.0)
            # transposed attention: compute K^T Q so softmax+AV need no transposes;
            # normalization folded into the output as a row-broadcast multiply
            for h in (range(NH) if 'attn' not in SKIP else []):
                kc = (h * DH) // 128
                ko = (h * DH) % 128
                exts = []
                for (k0, kn) in QT:
                    ps = ppool.tile([128, NQ], F32, tag="ps", name="ps")
                    nc.tensor.matmul(ps[:kn, :], qk_sl("k", h, slice(k0, k0 + kn)),
                                     qk_sl("q", h, slice(None)), start=True, stop=True)
                    ex = tmp.tile([128, NQ], BF16, tag="sattn", name="ex", bufs=3)
                    # logits bounded (|x| < 2 for this model's scale); exp directly
                    nc.scalar.activation(out=ex[:kn, :], in_=ps[:kn, :], func=ACT.Exp)
                    exts.append((ex, k0, kn))
                ps_sum = ppool.tile([1, NQ], F32, tag="ps", name="ps_sum")
                for ci, (ex, k0, kn) in enumerate(exts):
                    nc.tensor.matmul(ps_sum[:, :], ones_col[:kn, :], ex[:kn, :],
                                     start=(ci == 0), stop=(ci == 2))
                rc = tmp.tile([1, NQ], BF16, tag="srow", bufs=3)
                with nc.allow_low_precision(reason="bf16 softmax scale, tol 2e-2"):
                    nc.vector.reciprocal(out=rc[:, :], in_=ps_sum[:, :])
                rcb = ppool.tile([DH, NQ], F32, tag="pst", name="rcb", bufs=2)
                nc.tensor.matmul(rcb[:, :], ones_row[:, :DH], rc[:, :], start=True, stop=True)
                rcs = tmp.tile([DH, NQ], BF16, tag="rcs", name="rcs", bufs=2)
                nc.scalar.activation(out=rcs[:, :], in_=rcb[:, :], func=ACT.Identity)
                ps_o = ppool.tile([DH, NQ], F32, tag="pst", name="ps_o", bufs=2)
                for ci, (ex, k0, kn) in enumerate(exts):
                    nc.tensor.matmul(ps_o[:, :], v_nat[ci][:kn, h * DH:(h + 1) * DH], ex[:kn, :],
                                     start=(ci == 0), stop=(ci == 2))
                nc.vector.tensor_tensor(out=OT[ko:ko + DH, kc, :], in0=ps_o[:, :],
                                        in1=rcs[:, :], op=ALU.mult)

            t2 = proj_T(OT, sb_wo, sb_wob)
            x1 = newact()
            for kk in range(2):
                nc.vector.tensor_tensor(out=x1[:, kk, :], in0=sb_tgt[:, kk, :], in1=t2[:, kk, :], op=ALU.add)
            x1n = layer_norm(x1, 0, 1, newact())  # norm2

            # ================= deformable attention =================
            # packed value table: u32 element (pixel) = 2 bf16 channel-lanes;
            # partition p = head p//16, lanes = channels (p//16)*32 + (p%16)*2 + lane
            # one tile per level so gathers can start before the whole table is built
            LVB = [(0, 15002), (15000, 3752), (18750, 952), (19700, 250)]
            LSIZE = [15000, 3750, 950, 247]
            LBASE = [0, 15000, 18750, 19700]
            U32 = mybir.dt.uint32
            vtabs = [single.tile([128, LVB[l][1]], U32, name=f"vt{l}") for l in range(NL)]
            for l in range(NL):
                nc.vector.memset(vtabs[l][:, LSIZE[l]:], 0)
            MCH = 1024
            if 'value' in SKIP:
                for l in range(NL):
                    nc.vector.memset(vtabs[l][:, :], 0)
            def vtab_lane(lv, a, b, lane):
                return vtabs[lv][:, a:b].bitcast(BF16).rearrange("p (a b) -> p a b", b=2)[:, :, lane]
            for lv in ([1, 2, 3, 0] if 'value' not in SKIP else []):
                lsz = LSIZE[lv]
                for mi in range((lsz + MCH - 1) // MCH):
                    s0 = mi * MCH
                    sn = min(MCH, lsz - s0)
                    mt_ = mpool.tile([128, 2, MCH], BF16, tag="mem", name="mt_", bufs=2)
                    mv = mt_[:, :, :sn]
                    nc.sync.dma_start(out=mv,
                                      in_=bass.AP(tensor=memTb, offset=LBASE[lv] + s0,
                                                  ap=[[S, 128], [128 * S, 2], [1, sn]]))
                    for sub in range((sn + 511) // 512):
                        c0 = sub * 512
                        cn = min(512, sn - c0)
                        for lane in range(2):
                            ps = vpool.tile([128, 1024], F32, tag="vwps", name="vps")
                            for kk in range(2):
                                nc.tensor.matmul(ps[:, :cn], sb_wvd[:, kk, lane * 128:(lane + 1) * 128],
                                                 mt_[:, kk, c0:c0 + cn], start=(kk == 0), stop=(kk == 1))
                            r3 = (mi * 4 + sub * 2 + lane) % 2
                            if r3 == 0:
                                nc.scalar.activation(out=vtab_lane(lv, s0 + c0, s0 + c0 + cn, lane), in_=ps[:, :cn],
                                                     func=ACT.Identity, bias=sb_vdb[:, lane, :])
                            else:
                                nc.vector.tensor_scalar(out=vtab_lane(lv, s0 + c0, s0 + c0 + cn, lane),
                                                        in0=ps[:, :cn], scalar1=sb_vdb[:, lane, :],
                                                        scalar2=None, op0=ALU.add)

            q2 = newact()
            for kk in range(2):
                nc.vector.tensor_tensor(out=q2[:, kk, :], in0=x1n[:, kk, :], in1=sb_pos[:, kk, :], op=ALU.add)

            zt = single.tile([128, 160], F32, name="zt")
            nc.vector.memset(zt[:, :], 0.0)
            zp = zt[:, :].bitcast(BF16)
            nc.sync.dma_start(out=bass.AP(tensor=wdram, offset=0, ap=[[1280, 128], [1, 1280]]),
                              in_=bass.AP(tensor=zp.tensor, offset=zp.offset,
                                          ap=[list(zp.ap[0]), [0, 4], [1, 320]]))
            I16 = mybir.dt.int16
            idxs = single.tile([128, 1280], I16)
            ji = [single.tile([128, 304], I16, name=f"ji{m}") for m in range(4)]
            for m in range(4):
                nc.vector.memset(ji[m][:, :], 0)
            zu = zt[:, :].bitcast(I16)
            nc.sync.dma_start(out=bass.AP(tensor=jdram, offset=0, ap=[[1280, 128], [1, 1280]]),
                              in_=bass.AP(tensor=zu.tensor, offset=zu.offset,
                                          ap=[list(zu.ap[0]), [0, 4], [1, 320]]))
            if 'samp' in SKIP:
                pass
            for ti, (q0, qn) in (list(enumerate(QT)) if 'samp' not in SKIP else []):
                rt = tmp.tile([128, 8], F32, tag="refs", name="rt", bufs=1)
                nc.sync.dma_start(out=rt[:qn, :], in_=refs[q0:q0 + qn, :])
                ps_off = ppool.tile([128, D], F32, tag="ps")
                for kk in range(2):
                    nc.tensor.matmul(ps_off[:qn, :], _r(q2[:, kk, q0:q0 + qn]), _r(sb_wso[:, kk, :]),
                                     start=(kk == 0), stop=(kk == 1))
                off = tmp.tile([128, D], F32, tag="off", name="off", bufs=1)
                nc.vector.tensor_tensor(out=off[:qn, :], in0=ps_off[:qn, :], in1=SOBr[:qn, :], op=ALU.add)
                ps_aw = ppool.tile([128, 128], F32, tag="pst", name="ps_aw", bufs=2)
                for kk in range(2):
                    nc.tensor.matmul(ps_aw[:qn, :], _r(q2[:, kk, q0:q0 + qn]), _r(sb_waw[:, kk, :]),
                                     start=(kk == 0), stop=(kk == 1))
                awl = tmp.tile([128, 128], F32, tag="aw")
                nc.vector.tensor_tensor(out=awl[:qn, :], in0=ps_aw[:qn, :], in1=AWBr[:qn, :], op=ALU.add)
                # aw logits bounded (|x| < 3 for this model's scale); exp directly
                nc.scalar.activation(out=awl[:qn, :], in_=awl[:qn, :], func=ACT.Exp)
                sm = tmp.tile([128, NH], F32, tag="awrow", bufs=3)
                nc.vector.tensor_reduce(out=sm[:qn, :], in_=awl[:qn, :].rearrange("p (h g) -> p h g", g=16),
                                        op=ALU.add, axis=AX.X)
                rc = tmp.tile([128, NH], F32, tag="awrow", bufs=3)
                nc.vector.reciprocal(out=rc[:qn, :], in_=sm[:qn, :])
                rca = rc[:qn, :]
                aw = tmp.tile([128, 128], F32, tag="aw")
                nc.vector.tensor_tensor(out=aw[:qn, :].rearrange("p (h g) -> p h g", g=16),
                                        in0=awl[:qn, :].rearrange("p (h g) -> p h g", g=16),
                                        in1=bass.AP(tensor=rca.tensor, offset=rca.offset,
                                                    ap=[list(rca.ap[0]), list(rca.ap[1]), [0, 16]]),
                                        op=ALU.mult)

                J = tmp.tile([128, 512], F32, tag="J", name="J", bufs=1)
                Wt = tmp.tile([128, 512], BF16, tag="Wt", name="Wt", bufs=1)
                ra = rt[:qn, :]
                p_ = tmp.tile([128, 256], F32, tag="pxy", name="p_", bufs=2)
                for xy in range(2):
                    refb = bass.AP(tensor=ra.tensor, offset=ra.offset + xy,
                                   ap=[list(ra.ap[0]), [0, NH], [2, NL], [0, NP]])
                    nc.vector.tensor_tensor(
                        out=p_[:qn, xy * 128:(xy + 1) * 128].rearrange("p (h l m) -> p h l m", h=NH, l=NL),
                        in0=refb,
                        in1=SCLr[:qn, xy * 128:(xy + 1) * 128].rearrange("p (h l m) -> p h l m", h=NH, l=NL),
                        op=ALU.mult)
                nc.vector.tensor_tensor(out=p_[:qn, :], in0=p_[:qn, :], in1=off[:qn, :], op=ALU.add)
                # shift +64 so trunc == floor; consts pre-shifted on host
                nc.vector.tensor_scalar(out=p_[:qn, :], in0=p_[:qn, :], scalar1=63.5, scalar2=None, op0=ALU.add)
                xi = tmp.tile([128, 256], mybir.dt.int32, tag="scr", name="xi", bufs=2)
                nc.vector.tensor_copy(out=xi[:qn, :], in_=p_[:qn, :])
                x0 = tmp.tile([128, 256], F32, tag="x0m", name="x0", bufs=1)
                nc.vector.tensor_copy(out=x0[:qn, :], in_=xi[:qn, :])
                # cast may trunc or round-to-nearest; fix up to floor either way
                gt_ = tmp.tile([128, 256], F32, tag="scr", name="gt_", bufs=2)
                nc.vector.tensor_tensor(out=gt_[:qn, :], in0=x0[:qn, :], in1=p_[:qn, :], op=ALU.is_gt)
                nc.vector.tensor_tensor(out=x0[:qn, :], in0=x0[:qn, :], in1=gt_[:qn, :], op=ALU.subtract)
                nc.vector.tensor_scalar(out=x0[:qn, :], in0=x0[:qn, :], scalar1=64.0, scalar2=None, op0=ALU.max)
                nc.vector.tensor_tensor(out=x0[:qn, :], in0=x0[:qn, :], in1=CM2r[:qn, :], op=ALU.min)
                w0 = tmp.tile([128, 256], F32, tag="w0m", name="w0", bufs=1)
                w1_ = tmp.tile([128, 256], F32, tag="w1m", name="w1_", bufs=1)
                dt_ = tmp.tile([128, 256], F32, tag="scr", name="dt_", bufs=2)
                nc.vector.tensor_tensor(out=dt_[:qn, :], in0=p_[:qn, :], in1=x0[:qn, :], op=ALU.subtract)
                ab0 = tmp.tile([128, 256], F32, tag="scr", name="ab0", bufs=2)
                nc.scalar.activation(out=ab0[:qn, :], in_=dt_[:qn, :], func=ACT.Abs)
                nc.vector.tensor_scalar(out=ab0[:qn, :], in0=ab0[:qn, :], scalar1=-1.0, scalar2=1.0,
                                        op0=ALU.mult, op1=ALU.add)
                nc.vector.tensor_scalar(out=w0[:qn, :], in0=ab0[:qn, :], scalar1=0.0, scalar2=None, op0=ALU.max)
                nc.vector.tensor_scalar(out=dt_[:qn, :], in0=dt_[:qn, :], scalar1=-1.0, scalar2=None, op0=ALU.add)
                nc.scalar.activation(out=ab0[:qn, :], in_=dt_[:qn, :], func=ACT.Abs)
                nc.vector.tensor_scalar(out=ab0[:qn, :], in0=ab0[:qn, :], scalar1=-1.0, scalar2=1.0,
                                        op0=ALU.mult, op1=ALU.add)
                nc.vector.tensor_scalar(out=w1_[:qn, :], in0=ab0[:qn, :], scalar1=0.0, scalar2=None, op0=ALU.max)
                (xx0, wx0, wx1) = (x0[:, :128], w0[:, :128], w1_[:, :128])
                (yy0, wy0, wy1) = (x0[:, 128:], w0[:, 128:], w1_[:, 128:])
                jb = tmp.tile([128, 128], F32, tag="jb", name="jb", bufs=1)
                nc.vector.tensor_tensor(out=jb[:qn, :], in0=yy0[:qn, :], in1=WLr[:qn, :], op=ALU.mult)
                nc.vector.tensor_tensor(out=jb[:qn, :], in0=jb[:qn, :], in1=xx0[:qn, :], op=ALU.add)
                nc.vector.tensor_tensor(out=jb[:qn, :], in0=jb[:qn, :], in1=LSIr[:qn, :], op=ALU.add)
                # J col = h*64 + p*16 + c*8 + l*2 + pix; jb cols are (h:16, l:4, p:1)
                ja = J[:qn, :]
                def jview(c, x):
                    return bass.AP(tensor=ja.tensor, offset=ja.offset + c * 8 + x,
                                   ap=[list(ja.ap[0]), [64, 8], [2, 4], [16, 4]])
                nc.vector.tensor_copy(out=jview(0, 0), in_=jb[:qn, :])
                nc.vector.tensor_tensor(out=jview(1, 0), in0=jb[:qn, :], in1=WLr[:qn, :], op=ALU.add)
                nc.vector.tensor_scalar(out=jview(0, 1), in0=jview(0, 0), scalar1=1.0, scalar2=None,
                                        op0=ALU.add)
                nc.vector.tensor_scalar(out=jview(1, 1), in0=jview(1, 0), scalar1=1.0, scalar2=None,
                                        op0=ALU.add)
                t_c = []
                for c, wyc in ((0, wy0), (1, wy1)):
                    tc_ = tmp.tile([128, 128], F32, tag=f"tc{c}", name="tc_", bufs=1)
                    nc.vector.tensor_tensor(out=tc_[:qn, :], in0=aw[:qn, :], in1=wyc[:qn, :], op=ALU.mult)
                    t_c.append(tc_)
                Wv4 = Wt[:qn, :].rearrange("p (f c s) -> p f c s", c=2, s=2)
                for c in range(2):
                    for sp, wxv in ((0, wx0), (1, wx1)):
                        nc.vector.tensor_tensor(out=Wv4[:, :, c, sp], in0=t_c[c][:qn, :], in1=wxv[:qn, :],
                                                op=ALU.mult)
                wa = Wt[:qn, :]
                nc.sync.dma_start(
                    out=bass.AP(tensor=wdram, offset=q0 * 16,
                                ap=[[16, qn], [WQPAD * 64, NH], [5120, 4], [4, 4], [2, 2], [1, 2]]),
                    in_=bass.AP(tensor=wa.tensor, offset=wa.offset,
                                ap=[list(wa.ap[0]), [64, NH], [16, 4], [4, 4], [2, 2], [1, 2]]))
                for m in range(4):
                    pst = ppool.tile([128, 128], F32, tag="pst", name="pst", bufs=2)
                    nc.tensor.transpose(pst[:, :qn], J[:qn, m * 128:(m + 1) * 128], sb_ident[:qn, :qn])
                    pv = pst[:, :qn].rearrange("p (a b) -> p a b", b=2)
                    nc.vector.tensor_copy(out=ji[m][:, q0 // 2:q0 // 2 + qn // 2], in_=pv[:, :, 0])
                    nc.vector.tensor_copy(out=ji[m][:, 152 + q0 // 2:152 + q0 // 2 + qn // 2], in_=pv[:, :, 1])

            for m in range(4):
                # ji[m] rows = (h2, p, c, l, pix); jdram elem = P*1280 + lv*320 + qh*2 + pix,
                # P = (2m+h2)*16 + qm2*8 + p*2 + c; per-pix slices collapse to <=3 dims
                jm = ji[m][:, :]
                pp_ = jm.ap[0][0]
                for qm2 in range(2):
                    for pix in range(2):
                        eng = nc.sync if (m % 2 == 0) else nc.scalar
                        eng.dma_start(
                            out=bass.AP(tensor=jdram,
                                        offset=m * 2 * 20480 + qm2 * 10240 + pix * 160,
                                        ap=[[20480, 2], [320, 32], [1, 152]]),
                            in_=bass.AP(tensor=jm.tensor,
                                        offset=jm.offset + pix * pp_ + qm2 * 152,
                                        ap=[[2 * pp_, 64], [1, 152]]))
            iap = idxs[:, :]
            nc.sync.dma_start(
                out=bass.AP(tensor=iap.tensor, offset=iap.offset, ap=[list(iap.ap[0]), [1, 1280]]),
                in_=bass.AP(tensor=jdram, offset=0, ap=[[1, 163840]]))

            wsb = single.tile([NH * NSEL, SUBW], BF16)
            nc.sync.dma_start(out=wsb[:, :],
                              in_=bass.AP(tensor=wdram, offset=0, ap=[[WQPAD * 64, NH], [SUBW, NSEL], [1, SUBW]]))

            ODl = [actp.tile([128, 320, 2], BF16, name=f"OD_{l}") for l in range(NL)]
            # per level: 2 ap_gather calls (qh 0..96, 96..160); idx stream per 16-group:
            # i = qh*32 + pix*16 + (qm2*8 + p*2 + c); gathered u32 = 2 bf16 ch-lanes
            # per level: 2 ap_gather calls (pix=0: x0 pixels, pix=1: x0+1);
            # stream i = qh*16 + qm2*8 + p*2 + c per 16-group; u32 = 2 bf16 ch-lanes
            for lv in ([1, 2, 3, 0] if 'gather' not in SKIP else []):
                b0, blen = LVB[lv]
                gts = {}
                for pix in range(2):
                    gt = gpool.tile([128, 2560], mybir.dt.uint32, tag="gather", name="gt", bufs=2)
                    nc.gpsimd.ap_gather(out_ap=gt[:, :], in_ap=vtabs[lv][:, :],
                                        idxs_ap=idxs[:, lv * 320 + pix * 160:lv * 320 + pix * 160 + 160],
                                        channels=128, num_elems=blen, d=1, num_idxs=2560)
                    gts[pix] = gt
                for gi in range(5):
                    sl0 = gi * 32
                    psw = vpool.tile([128, 1024], F32, tag="vwps", name="psw")
                    for half in range(2):
                        el0 = lv * 5120 + (sl0 + half * 16) * 32
                        sub = el0 // SUBW
                        eoff = el0 % SUBW
                        nc.tensor.matmul(psw[:, half * 512:(half + 1) * 512],
                                         sb_sel[:, sub * 128:(sub + 1) * 128],
                                         wsb[:, eoff:eoff + 512], start=True, stop=True)
                    prod = mpool.tile([128, 2048], BF16, tag="prod", name="prod", bufs=2)
                    pr = prod[:, :]
                    pa = psw[:, :]
                    for pix in range(2):
                        gh = gts[pix][:, :].bitcast(BF16)  # (i, lane)
                        # iteration (qh32, qm2, p, c, lane)
                        ga = bass.AP(tensor=gh.tensor, offset=gh.offset + sl0 * 32,
                                     ap=[list(gh.ap[0]), [32, 32], [16, 2], [4, 4], [2, 2], [1, 2]])
                        pb = bass.AP(tensor=pa.tensor, offset=pa.offset + pix,
                                     ap=[list(pa.ap[0]), [32, 32], [16, 2], [4, 4], [2, 2], [0, 2]])
                        # prod layout: qh*64 + qm2*32 + lane*16 + p*4 + c*2 + pix
                        pw = bass.AP(tensor=pr.tensor, offset=pr.offset + pix,
                                     ap=[list(pr.ap[0]), [64, 32], [32, 2], [4, 4], [2, 2], [16, 2]])
                        nc.vector.tensor_tensor(out=pw, in0=ga, in1=pb, op=ALU.mult)
                    # reduce over contiguous (p,c,pix)=16 keeping (qh, qm2, lane)
                    ra = bass.AP(tensor=pr.tensor, offset=pr.offset,
                                 ap=[list(pr.ap[0]), [64, 32], [32, 2], [16, 2], [1, 16]])
                    ov = ODl[lv][:, sl0 * 2:sl0 * 2 + 64, :]
                    ow = bass.AP(tensor=ov.tensor, offset=ov.offset,
                                 ap=[list(ov.ap[0]), [4, 32], [2, 2], [1, 2]])
                    with nc.allow_low_precision(reason="16-term bf16 reduce, tol 2e-2"):
                        nc.vector.tensor_reduce(out=ow, in_=ra, op=ALU.add, axis=AX.X)
            if 'gather' in SKIP:
                for lv in range(NL):
                    nc.vector.memset(ODl[lv][:, :, :], 0.0)
            t2d = newact()
            for mt in range(2):
                ps = ppool.tile([128, NQ], F32, tag="ps")
                for ci, (lane, lv) in enumerate([(l_, v_) for l_ in range(2) for v_ in range(NL)]):
                    nc.tensor.matmul(ps[:, :], sb_wod[:, lane, mt * 128:(mt + 1) * 128],
                                     ODl[lv][:, :NQ, lane], start=(ci == 0), stop=(ci == 7))
                nc.vector.tensor_scalar(out=t2d[:, mt, :], in0=ps[:, :], scalar1=sb_wodb[:, mt, :],
                                        scalar2=None, op0=ALU.add)
            x2 = newact()
            for kk in range(2):
                nc.vector.tensor_tensor(out=x2[:, kk, :], in0=x1n[:, kk, :], in1=t2d[:, kk, :], op=ALU.add)
            x2n = layer_norm(x2, 2, 3, newact())  # norm1

            # ================= FFN =================
            h1 = actp.tile([128, 8, NQ], BF16)
            if 'ffn' in SKIP:
                for mt in range(8):
                    nc.vector.memset(h1[:, mt, :], 0.0)
            for mt in (range(8) if 'ffn' not in SKIP else []):
                ps = ppool.tile([128, NQ], F32, tag="ps")
                for kk in range(2):
                    nc.tensor.matmul(ps[:, :], sb_w1[:, kk, mt * 128:(mt + 1) * 128], x2n[:, kk, :],
                                     start=(kk == 0), stop=(kk == 1))
                nc.scalar.activation(out=h1[:, mt, :], in_=ps[:, :], func=ACT.Relu, bias=sb_fsm[:, 0, 12 + mt:13 + mt])
            t2f = newact()
            for mt in range(2):
                ps = ppool.tile([128, NQ], F32, tag="ps")
                for kk in range(8):
                    nc.tensor.matmul(ps[:, :], sb_w2[:, kk, mt * 128:(mt + 1) * 128], h1[:, kk, :],
                                     start=(kk == 0), stop=(kk == 7))
                nc.vector.tensor_scalar(out=t2f[:, mt, :], in0=ps[:, :], scalar1=sb_b2[:, mt, :],
                                        scalar2=None, op0=ALU.add)
            x3 = newact()
            for kk in range(2):
                nc.vector.tensor_tensor(out=x3[:, kk, :], in0=x2n[:, kk, :], in1=t2f[:, kk, :], op=ALU.add)
            x3n = layer_norm(x3, 4, 5, actp.tile([128, 2, NQ], F32, name="actsf"))  # norm3
            for kk in range(2):
                nc.sync.dma_start(out=outT[kk * 128:(kk + 1) * 128, :], in_=x3n[:, kk, :])

    nc.compile()
    return nc


def _perm_so():
    # samp_off_w rows are (h, l, p, xy); reorder to (xy, h, l, p)
    return np.array([((h * NL + l) * NP + p) * 2 + xy
                     for xy in range(2) for h in range(NH) for l in range(NL) for p in range(NP)])


def _host_prep(inputs):
    import ml_dtypes
    f = lambda x: np.ascontiguousarray(np.asarray(x, dtype=np.float32))
    bf = lambda x: np.ascontiguousarray(np.asarray(x, np.float32).astype(ml_dtypes.bfloat16))
    in_w = f(inputs["in_proj_w"]); in_b = f(inputs["in_proj_b"])
    qw, kw, vw = in_w[:D], in_w[D:2 * D], in_w[2 * D:]
    qb_, kb_, vb_ = in_b[:D], in_b[D:2 * D], in_b[2 * D:]
    sc = 1.0 / np.sqrt(DH)
    perm = np.array([(p // 16) * 32 + (p % 16) * 2 + lane
                     for lane in range(2) for p in range(128)])
    # bf16 weight pack [D, 2944]
    wbig = np.concatenate([
        (qw * sc).T, kw.T, vw.T,                       # 0:256, 256:512, 512:768
        f(inputs["out_proj_w"]).T,                     # 768:1024
        f(inputs["samp_off_w"])[_perm_so()].T,         # 1024:1280
        f(inputs["attn_wt_w"]).T,                      # 1280:1408
        f(inputs["value_w"])[perm].T,                  # 1408:1664
        f(inputs["outp_w"]).T[perm],                   # 1664:1920
        f(inputs["lin1_w"]).T,                         # 1920:2944
    ], axis=1)
    # f32 smalls pack [D, 20]
    fsm = np.zeros((D, 20), np.float32)
    fsm[:, 0] = qb_ * sc; fsm[:, 1] = kb_
    fsm[:, 2] = f(inputs["out_proj_b"]); fsm[:, 3] = f(inputs["value_b"])[perm]
    fsm[:, 4] = f(inputs["outp_b"]); fsm[:, 5] = f(inputs["lin2_b"])
    fsm[:, 6] = np.concatenate([f(inputs["norm2_g"])[:128], f(inputs["norm2_g"])[128:]])
    for ci, nm in enumerate(["norm2_g", "norm2_b", "norm1_g", "norm1_b", "norm3_g", "norm3_b"]):
        fsm[:, 6 + ci] = f(inputs[nm])
    fsm[:128, 12:20] = f(inputs["lin1_b"]).reshape(8, 128).T
    shared = {"wbig": bf(wbig), "fsm": np.ascontiguousarray(fsm),
              "w2T": bf(f(inputs["lin2_w"]).T),
              "ident_in": np.eye(128, dtype=np.float32),
              "identb_in": bf(np.eye(128, dtype=np.float32))}
    Wv_ = SPATIAL[:, 1].astype(np.float32); Hv_ = SPATIAL[:, 0].astype(np.float32)
    row = lambda vals: np.tile(np.repeat(vals, NP), NH)
    lsi_adj = -64.0 * Wv_ - 64.0  # level-local indices
    pad128 = np.zeros(128, np.float32)
    shared["consts"] = np.ascontiguousarray(np.stack([
        np.concatenate([row(Wv_), row(Hv_)]),
        np.concatenate([row(Wv_ + 62), row(Hv_ + 62)]),
        np.concatenate([row(lsi_adj), pad128]),
        np.concatenate([f(inputs["attn_wt_b"]), pad128])]).astype(np.float32))
    shared["consts2"] = np.ascontiguousarray(
        np.stack([f(inputs["samp_off_b"])[_perm_so()], vb_]).astype(np.float32))
    sel = np.zeros((NH * NSEL, NSEL * 128), dtype=np.float32)
    for s_ in range(NSEL):
        for p in range(128):
            sel[(p // 16) * NSEL + s_, s_ * 128 + p] = 1.0
    shared["sel_in"] = bf(sel)
    per_core = []
    for b in range(BS):
        m = dict(shared)
        m["tpT"] = bf(np.concatenate([f(inputs["tgt"][b]).T, f(inputs["tgt_query_pos"][b]).T], axis=1))
        m["memTb"] = bf(f(inputs["memory"][b]).T)
        m["refs"] = np.ascontiguousarray(f(inputs["tgt_reference_points"][b]).reshape(NQ, 8))
        per_core.append(m)
    return per_core


def kernel(**inputs) -> np.ndarray:
    if "nc" not in _cache:
        _cache["nc"] = build_bass()
    nc = _cache["nc"]
    in_maps = _host_prep(inputs)
    res = run_bass_kernel_spmd(nc, in_maps, core_ids=list(range(BS)))
    out = np.stack([np.ascontiguousarray(r["outT"].T) for r in res.results])
    return out.astype(np.float32)

